# revision 1
# baseline (speedup 1.0000x reference)
"""Multi-head attention block (QKV proj + softmax attention + out-proj +
residual + LayerNorm) on 8 TRN2 NeuronCores.

Sharding: core = (batch b, token-half g). Each core computes attention for
its 1024 query tokens over all 8 heads (K/V over the full 2048 tokens of its
batch are recomputed per pair — cheaper than cross-core collectives), then
the output projection, residual and LayerNorm for its token half. Outputs
are disjoint [1024, 1024] shards concatenated on the host.

Inputs are token-rotated per core on the host so that rows 0..1023 of the
per-core `x` are always that core's query tokens (softmax over k is
permutation-invariant, so K/V built from the rotated order are fine). The
host also pre-transposes x to d-major bf16, so the kernel needs no
on-device transposes of x.

Matmuls run in bf16 (PE full rate + fast weight load); accumulation is
fp32 in PSUM, softmax statistics and LayerNorm are fp32. The attention
k-chunk loop is software-pipelined: scores+exp for chunk kc+1 issue ahead
of the PV/ones matmuls of chunk kc so the ScalarE exp latency is hidden.
"""

import contextlib
import sys

if '/opt/trn_rl_repo' not in sys.path:
    sys.path.insert(0, '/opt/trn_rl_repo')

import ml_dtypes
import numpy as np

import concourse.bacc as bacc
import concourse.bass as bass
import concourse.bass_utils as bass_utils
import concourse.tile as tile
from concourse import mybir
from concourse.masks import make_identity

B, T, D, H = 4, 2048, 1024, 8
DH = 128            # head dim
TQ = T // 2         # query tokens per core
N_CORES = 8
DC = D // 128       # d-chunks of 128
KC = T // 128       # k-token chunks of 128
QC = TQ // 128      # q-token chunks of 128
EPS = 1e-5
ISCALE = 1.0 / float(np.sqrt(DH))
F32 = mybir.dt.float32
BF16 = mybir.dt.bfloat16
AF = mybir.ActivationFunctionType
ALU = mybir.AluOpType
BF = ml_dtypes.bfloat16


def _body(nc, tc, ap, es, apply_gb):
    xq, xbT, Wq, bq, Wk, bk, Wv, bv, Wo, gamma, beta, y = (
        ap['xq'], ap['xbT'], ap['Wq'], ap['bq'], ap['Wk'], ap['bk'],
        ap['Wv'], ap['bv'], ap['Wo'], ap['gamma'], ap['beta'], ap['y'])

    consts = es.enter_context(tc.tile_pool(name="consts", bufs=1))
    ctx_pool = es.enter_context(tc.tile_pool(name="ctx", bufs=1))
    xt_pool = es.enter_context(tc.tile_pool(name="xt", bufs=1))
    w_pool = es.enter_context(tc.tile_pool(name="w", bufs=6))
    kt_pool = es.enter_context(tc.tile_pool(name="kt", bufs=2))
    vt_pool = es.enter_context(tc.tile_pool(name="vt", bufs=2))
    v_pool = es.enter_context(tc.tile_pool(name="v", bufs=2))
    qt_pool = es.enter_context(tc.tile_pool(name="qt", bufs=2))
    pt_pool = es.enter_context(tc.tile_pool(name="pt", bufs=4))
    sums_pool = es.enter_context(tc.tile_pool(name="sums", bufs=2))
    wo_pool = es.enter_context(tc.tile_pool(name="wo", bufs=1))
    xr_pool = es.enter_context(tc.tile_pool(name="xr", bufs=2))
    y3_pool = es.enter_context(tc.tile_pool(name="y3", bufs=2))
    ln_pool = es.enter_context(tc.tile_pool(name="ln", bufs=4))

    # ---- constants -------------------------------------------------------
    ident = consts.tile([128, 128], BF16, tag="ident")
    make_identity(nc, ident)
    ones = consts.tile([128, 1], BF16, tag="ones")
    nc.vector.memset(ones, 1.0)
    eps_t = consts.tile([128, 1], F32, tag="eps")
    nc.vector.memset(eps_t, EPS)


    # partition-broadcast rows (per-feature vectors used on the free dim)
    def bcast128(name, src):
        t = consts.tile([128, D], F32, tag=name, name=name)
        src_b = bass.AP(tensor=src.tensor, offset=src.offset,
                        ap=[[0, 128]] + src.ap)
        nc.sync.dma_start(out=t, in_=src_b)
        return t


    ctx = [ctx_pool.tile([128, TQ], BF16, tag=f"ctx{h}", name=f"ctx{h}")
           for h in range(H)]

    # x^T (d-major) comes pre-transposed from the host: straight DMA
    # loads, split across two DMA paths to shorten the startup ramp
    xt = [xt_pool.tile([128, T], BF16, tag=f"xt{dc}", name=f"xt{dc}")
          for dc in range(DC)]
    for dc in range(DC):
        nc.sync.dma_start(out=xt[dc], in_=xbT[dc * 128:(dc + 1) * 128, :])

    # per-head bias layout: bias_t[p, h] = b[h*128 + p]
    bq_t = consts.tile([128, H], F32, tag="bq")
    bk_t = consts.tile([128, 4], F32, tag="bk")
    bv_t = consts.tile([128, 4], F32, tag="bv")
    nc.sync.dma_start(out=bq_t, in_=bq.rearrange("(h p) -> p h", p=128))
    nc.sync.dma_start(out=bk_t, in_=bk.rearrange("(h p) -> p h", p=128))
    nc.sync.dma_start(out=bv_t, in_=bv.rearrange("(h p) -> p h", p=128))

    # Wo blocks: prefetched mid phase 2 (read only in phase 3)
    wo_t = [wo_pool.tile([128, D], BF16, tag=f"wo{dc}", name=f"wo{dc}")
            for dc in range(DC)]

    # ---- phase 2: local K/V + pair AllGather, then per-head attention ----
    with contextlib.ExitStack() as es2:
        wk_psum = es2.enter_context(tc.tile_pool(name="wk_ps", bufs=4,
                                                 space="PSUM"))
        ctx_psum = es2.enter_context(tc.tile_pool(name="ctx_ps", bufs=1,
                                                  space="PSUM"))
        sum_psum = es2.enter_context(tc.tile_pool(name="sum_ps", bufs=1,
                                                  space="PSUM"))
        dram = es2.enter_context(tc.tile_pool(name="dram", bufs=1,
                                              space="DRAM"))

        def proj_nt(dst, w_b, bias_col, nt):
            nsl = slice(nt * 512, (nt + 1) * 512)
            pp = wk_psum.tile([128, 512], F32, tag="ps", name="pp")
            for dc in range(DC):
                nc.tensor.matmul(pp, w_b[dc], xt[dc][:, nsl],
                                 start=(dc == 0), stop=(dc == DC - 1))
            nc.vector.tensor_scalar(out=dst[:, nsl], in0=pp,
                                    scalar1=bias_col, scalar2=None,
                                    op0=ALU.add)

        # --- stage A: K/V for this core's 4 local heads (host-permuted
        # Wk/Wv pick the right actual heads), exchanged with the pair
        # partner via AllGather; group order makes k_all/v_all canonical
        # (entry hh = actual head hh) on both cores.
        k_send = dram.tile([4, 128, T], BF16, tag="k_send")
        v_send = dram.tile([4, 128, T], BF16, tag="v_send")
        k_all = dram.tile([H, 128, T], BF16, tag="k_all")
        v_all = dram.tile([H, 128, T], BF16, tag="v_all")

        def dma_w_blocks(W, hsl, tag):
            blocks = []
            for dc in range(DC):
                dsl = slice(dc * 128, (dc + 1) * 128)
                wt = w_pool.tile([128, 128], BF16, tag=tag, name="wb")
                nc.sync.dma_start(out=wt, in_=W[dsl, hsl])
                blocks.append(wt)
            return blocks

        for j in range(4):
            jsl = slice(j * 128, (j + 1) * 128)
            wk_b = dma_w_blocks(Wk, jsl, f"wk{j % 2}")
            ktl = kt_pool.tile([128, T], BF16, tag="kt", name=f"ktl{j}")
            for nt in range(T // 512):
                proj_nt(ktl, wk_b, bk_t[:, j:j + 1], nt)
            nc.sync.dma_start(out=k_send[j], in_=ktl)
        nc.gpsimd.collective_compute(
            "AllGather", mybir.AluOpType.bypass,
            ins=[k_send.opt()], outs=[k_all.opt()],
            replica_groups=[[0, 1], [2, 3], [4, 5], [6, 7]])

        for j in range(4):
            jsl = slice(j * 128, (j + 1) * 128)
            wv_b = dma_w_blocks(Wv, jsl, f"wv{j % 2}")
            vtl = vt_pool.tile([128, T], BF16, tag="vt", name=f"vtl{j}")
            for nt in range(T // 512):
                proj_nt(vtl, wv_b, bv_t[:, j:j + 1], nt)
            vl = v_pool.tile([128, KC, 128], BF16, tag="v", name=f"vl{j}")
            for kc in range(KC):
                tp = wk_psum.tile([128, 128], BF16, tag="ps", name="tpv")
                nc.tensor.transpose(tp, vtl[:, kc * 128:(kc + 1) * 128],
                                    ident)
                nc.vector.tensor_copy(out=vl[:, kc, :], in_=tp)
            nc.sync.dma_start(out=v_send[j],
                              in_=vl.rearrange("p a b -> p (a b)"))
        nc.gpsimd.collective_compute(
            "AllGather", mybir.AluOpType.bypass,
            ins=[v_send.opt()], outs=[v_all.opt()],
            replica_groups=[[0, 1], [2, 3], [4, 5], [6, 7]])

        # --- per-head state: Q projection tasks + K/V fetch from the
        # gathered buffers (canonical head order, uniform across cores)
        def load_kv(h):
            kt = kt_pool.tile([128, T], BF16, tag="ktg", name=f"kt{h}",
                              bufs=3)
            nc.sync.dma_start(out=kt, in_=k_all[h])
            v = v_pool.tile([128, KC, 128], BF16, tag="vg", name=f"v{h}",
                            bufs=3)
            nc.sync.dma_start(out=v.rearrange("p a b -> p (a b)"),
                              in_=v_all[h])
            return kt, v

        def make_head_tasks(h):
            wq_b = dma_w_blocks(Wq, slice(h * 128, (h + 1) * 128),
                                f"wq{h % 2}")
            qt = qt_pool.tile([128, TQ], BF16, tag="qt", name=f"qt{h}")
            tasks = [lambda nt=nt: proj_nt(qt, wq_b, bq_t[:, h:h + 1], nt)
                     for nt in range(TQ // 512)]
            return {'qt': qt, 'tasks': tasks}

        kv_tiles = {0: load_kv(0), 1: load_kv(1)}

        # prologue: head 0 Q projection runs un-interleaved
        head_cur = make_head_tasks(0)
        for t in head_cur['tasks']:
            t()
        head_cur['tasks'] = []

        for h in range(H):
            if h + 1 < H:
                head_next = make_head_tasks(h + 1)
            else:
                head_next = None
            if h + 2 < H:
                kv_tiles[h + 2] = load_kv(h + 2)
            if h == 1:
                gb = [bcast128("gamma_b", gamma), bcast128("beta_b", beta)] \
                    if apply_gb else None
            if h == 2:
                for dc in range(DC):
                    nc.sync.dma_start(out=wo_t[dc],
                                      in_=Wo[dc * 128:(dc + 1) * 128, :])

            kt, v = kv_tiles.pop(h)
            qt = head_cur['qt']
            bg = list(head_next['tasks']) if head_next else []
            bg_i = 0

            # attention, software-pipelined over k-chunks; next head's
            # projection tasks are drip-fed between chunks to keep the PE
            # busy while ScalarE works through the exps
            ctx_ps = ctx_psum.tile([128, TQ], F32, tag="ctx_ps")
            sum_ps = sum_psum.tile([1, TQ], F32, tag="sum_ps")

            def scores_exp(kc):
                ks = slice(kc * 128, (kc + 1) * 128)
                pt = pt_pool.tile([128, TQ], BF16, tag="pt", name="pt")
                for nq in range(TQ // 512):
                    nsl = slice(nq * 512, (nq + 1) * 512)
                    s_ps = wk_psum.tile([128, 512], F32, tag="ps",
                                        name="s_ps")
                    nc.tensor.matmul(s_ps, kt[:, ks], qt[:, nsl],
                                     start=True, stop=True)
                    nc.scalar.activation(out=pt[:, nsl], in_=s_ps,
                                         func=AF.Exp, scale=ISCALE)
                return pt

            pt_cur = scores_exp(0)
            for kc in range(KC):
                pt_next = scores_exp(kc + 1) if kc + 1 < KC else None
                for nq in range(TQ // 512):
                    nsl = slice(nq * 512, (nq + 1) * 512)
                    nc.tensor.matmul(ctx_ps[:, nsl], v[:, kc, :],
                                     pt_cur[:, nsl],
                                     start=(kc == 0), stop=(kc == KC - 1))
                for nq in range(TQ // 512):
                    nsl = slice(nq * 512, (nq + 1) * 512)
                    nc.tensor.matmul(sum_ps[:, nsl], ones, pt_cur[:, nsl],
                                     start=(kc == 0), stop=(kc == KC - 1))
                if bg_i < len(bg):
                    bg[bg_i]()
                    bg_i += 1
                pt_cur = pt_next
            while bg_i < len(bg):
                bg[bg_i]()
                bg_i += 1

            # free the PSUM accumulators fast; normalize off the
            # critical path (reciprocal + broadcast + in-place scale)
            nc.vector.tensor_copy(out=ctx[h], in_=ctx_ps)
            ssb = sums_pool.tile([1, TQ], F32, tag="ssb")
            nc.vector.tensor_copy(out=ssb, in_=sum_ps)
            rsum = sums_pool.tile([1, TQ], F32, tag="rsum")
            nc.vector.reciprocal_approx_fast(out=rsum, in_=ssb)
            rsum_b = sums_pool.tile([128, TQ], F32, tag="rsum_b")
            nc.gpsimd.partition_broadcast(rsum_b, rsum, channels=128)
            nc.vector.tensor_mul(out=ctx[h], in0=ctx[h], in1=rsum_b)

            head_cur = head_next

    # ---- phase 3: out-projection + residual + LayerNorm ------------------
    with tc.tile_pool(name="y_ps", bufs=2, space="PSUM") as y_psum:
        for qc in range(QC):
            qs = slice(qc * 128, (qc + 1) * 128)
            y_ps = y_psum.tile([128, D], F32, tag="y_ps")
            for no in range(D // 512):
                nsl = slice(no * 512, (no + 1) * 512)
                for dc in range(DC):
                    nc.tensor.matmul(y_ps[:, nsl], ctx[dc][:, qs],
                                     wo_t[dc][:, nsl],
                                     start=(dc == 0), stop=(dc == DC - 1))

            xr = xr_pool.tile([128, D], F32, tag="xr")
            nc.sync.dma_start(out=xr, in_=xq[qc * 128:(qc + 1) * 128, :])
            y1 = y3_pool.tile([128, D], F32, tag="y1")
            nc.vector.tensor_add(out=y1, in0=y_ps, in1=xr)  # resid (+bo)

            # LayerNorm over the feature dim
            stats = ln_pool.tile([128, 2, 6], F32, tag="stats")
            y1g = y1.rearrange("p (n f) -> p n f", f=512)
            nc.vector.bn_stats(out=stats[:, 0, :], in_=y1g[:, 0, :])
            nc.vector.bn_stats(out=stats[:, 1, :], in_=y1g[:, 1, :])
            mv = ln_pool.tile([128, 2], F32, tag="mv")
            nc.vector.bn_aggr(out=mv, in_=stats)
            std = ln_pool.tile([128, 1], F32, tag="std")
            nc.scalar.activation(out=std, in_=mv[:, 1:2], func=AF.Sqrt,
                                 bias=eps_t)
            rstd = ln_pool.tile([128, 1], F32, tag="rstd")
            nc.vector.reciprocal(out=rstd, in_=std)
            y2 = y3_pool.tile([128, D], F32, tag="y2")
            nc.vector.tensor_scalar(out=y2, in0=y1, scalar1=mv[:, 0:1],
                                    scalar2=rstd, op0=ALU.subtract,
                                    op1=ALU.mult)
            if apply_gb:
                nc.vector.tensor_mul(out=y2, in0=y2, in1=gb[0])
                nc.vector.tensor_add(out=y2, in0=y2, in1=gb[1])
            nc.sync.dma_start(out=y[qs, :], in_=y2)


def build(apply_gb=True):
    nc = bacc.Bacc("TRN2", target_bir_lowering=False, debug=False,
                   enable_asserts=False, num_devices=N_CORES)
    ap = {}
    ap['xq'] = nc.dram_tensor("xq", [TQ, D], F32, kind="ExternalInput").ap()
    ap['xbT'] = nc.dram_tensor("xbT", [D, T], BF16, kind="ExternalInput").ap()
    ap['Wq'] = nc.dram_tensor("Wq", [D, D], BF16, kind="ExternalInput").ap()
    ap['bq'] = nc.dram_tensor("bq", [D], F32, kind="ExternalInput").ap()
    ap['Wo'] = nc.dram_tensor("Wo", [D, D], BF16, kind="ExternalInput").ap()
    ap['Wk'] = nc.dram_tensor("Wk", [D, 512], BF16,
                              kind="ExternalInput").ap()
    ap['bk'] = nc.dram_tensor("bk", [512], F32, kind="ExternalInput").ap()
    ap['Wv'] = nc.dram_tensor("Wv", [D, 512], BF16,
                              kind="ExternalInput").ap()
    ap['bv'] = nc.dram_tensor("bv", [512], F32, kind="ExternalInput").ap()
    ap['gamma'] = nc.dram_tensor("gamma", [D], F32, kind="ExternalInput").ap()
    ap['beta'] = nc.dram_tensor("beta", [D], F32, kind="ExternalInput").ap()
    ap['y'] = nc.dram_tensor("y", [TQ, D], F32, kind="ExternalOutput").ap()

    with tile.TileContext(nc) as tc, contextlib.ExitStack() as es:
        _body(nc, tc, ap, es, apply_gb)
    nc.compile()
    return nc


def make_in_maps(inputs):
    """Per-core input maps; x token-rotated so q tokens come first."""
    f32 = {k: np.ascontiguousarray(np.asarray(v, dtype=np.float32))
           for k, v in inputs.items()}
    shared = {k: f32[k] for k in ('bq', 'gamma', 'beta')}
    for w in ('Wq', 'Wo'):
        shared[w] = np.ascontiguousarray(f32[w].astype(BF))
    wk_bf = f32['Wk'].astype(BF)
    wv_bf = f32['Wv'].astype(BF)
    x = f32['x']
    in_maps = []
    for core in range(N_CORES):
        b, g = divmod(core, 2)
        own = slice(512 * g, 512 * (g + 1))
        xr = np.roll(x[b], -TQ * g, axis=0)
        in_maps.append({'xq': np.ascontiguousarray(xr[:TQ] + f32['bo']),
                        'xbT': np.ascontiguousarray(xr.T.astype(BF)),
                        'Wk': np.ascontiguousarray(wk_bf[:, own]),
                        'bk': f32['bk'][own].copy(),
                        'Wv': np.ascontiguousarray(wv_bf[:, own]),
                        'bv': f32['bv'][own].copy(),
                        **shared})
    return in_maps


_NC = None
_NC_GB = None


def kernel(**inputs):
    global _NC, _NC_GB
    apply_gb = not (np.all(np.asarray(inputs['gamma']) == 1.0)
                    and np.all(np.asarray(inputs['beta']) == 0.0))
    if _NC is None or _NC_GB != apply_gb:
        _NC = build(apply_gb)
        _NC_GB = apply_gb
    in_maps = make_in_maps(inputs)
    res = bass_utils.run_bass_kernel_spmd(_NC, in_maps,
                                          core_ids=list(range(N_CORES)))
    out = np.empty((B, T, D), dtype=np.float32)
    for core in range(N_CORES):
        b, g = divmod(core, 2)
        out[b, TQ * g:TQ * (g + 1)] = res.results[core]['y']
    return out



# revision 8
# speedup vs baseline: 1.4466x; 1.4466x over previous
"""Multi-head attention block (QKV proj + softmax attention + out-proj +
residual + LayerNorm) on 8 TRN2 NeuronCores.

Sharding: core = (batch b, token-half g). Each core computes attention for
its 1024 query tokens over all 8 heads. K/V for the core's 4 local heads
are computed over the full 2048 tokens and exchanged with the pair partner
via AllGather; the gather latency hides under the V/Q projections.

Precision: weights are host-scaled by 8 and cast to fp8e4 (dodges fp8
subnormals; compensated exactly: exp scale /64 for Q*K, ones=8 for the
softmax denominator, LayerNorm scale-invariance with eps*64 for the
residual path). Matmuls with contraction >=256 run fp8 DoubleRow (2
contraction rows per pass); scores run bf16 (contraction = head dim 128).
The V projection runs with x as the stationary operand, producing
v[token, dh] directly (no PE transposes). The residual is injected into
the out-projection PSUM via an identity matmul. Accumulation is f32 in
PSUM; softmax statistics and LayerNorm are f32.
"""

import contextlib
import sys

if '/opt/trn_rl_repo' not in sys.path:
    sys.path.insert(0, '/opt/trn_rl_repo')

import ml_dtypes
import numpy as np

import concourse.bacc as bacc
import concourse.bass as bass
import concourse.bass_utils as bass_utils
import concourse.tile as tile
from concourse import mybir
from concourse.masks import make_identity

B, T, D, H = 4, 2048, 1024, 8
DH = 128            # head dim
TQ = T // 2         # query tokens per core
N_CORES = 8
DC = D // 128       # d-chunks of 128
KC = T // 128       # k-token chunks of 128
QC = TQ // 128      # q-token chunks of 128
EPS = 1e-5
WS = 8.0            # host-side weight scale (keeps fp8 weights normal)
SC_EXP = 1.0 / (float(np.sqrt(DH)) * WS * WS)
F32 = mybir.dt.float32
BF16 = mybir.dt.bfloat16
FP8 = mybir.dt.float8e4
AF = mybir.ActivationFunctionType
ALU = mybir.AluOpType
DR = mybir.MatmulPerfMode.DoubleRow
BF = ml_dtypes.bfloat16
E4 = ml_dtypes.float8_e4m3


def _body(nc, tc, ap, es, apply_gb):
    xt8, xq8, Wq, bq, Wk, bk, Wv, bv, Wo, gamma, beta, y = (
        ap['xt8'], ap['xq8'], ap['Wq'], ap['bq'], ap['Wk'], ap['bk'],
        ap['Wv'], ap['bv'], ap['Wo'], ap['gamma'], ap['beta'], ap['y'])

    consts = es.enter_context(tc.tile_pool(name="consts", bufs=1))
    w_pool = es.enter_context(tc.tile_pool(name="w", bufs=1))
    krem_pool = es.enter_context(tc.tile_pool(name="krem", bufs=3))
    vrem_pool = es.enter_context(tc.tile_pool(name="vrem", bufs=3))
    qt_pool = es.enter_context(tc.tile_pool(name="qt", bufs=1))
    pt_pool = es.enter_context(tc.tile_pool(name="pt", bufs=2))
    cb_pool = es.enter_context(tc.tile_pool(name="cb", bufs=2))
    sums_pool = es.enter_context(tc.tile_pool(name="sums", bufs=2))
    y2_pool = es.enter_context(tc.tile_pool(name="y2", bufs=2))
    xq_pool = es.enter_context(tc.tile_pool(name="xq", bufs=1))
    dram = es.enter_context(tc.tile_pool(name="dram", bufs=1, space="DRAM"))

    # ---- weight / x loads (issue order = DMA priority) -------------------
    wk_t = w_pool.tile([128, DC, 512], FP8, tag="wk")
    nc.sync.dma_start(out=wk_t, in_=Wk)
    xt = w_pool.tile([128, DC, T], FP8, tag="xt")
    for tb in range(4):
        tsl = slice(tb * 512, (tb + 1) * 512)
        nc.sync.dma_start(out=xt[:, :, tsl], in_=xt8[:, :, tsl])
    wv_t = w_pool.tile([128, DC, 512], FP8, tag="wv")
    nc.sync.dma_start(out=wv_t, in_=Wv)
    wq_t = w_pool.tile([128, DC, D], FP8, tag="wq")
    nc.sync.dma_start(out=wq_t, in_=Wq)

    ident = consts.tile([128, 128], BF16, tag="ident")
    make_identity(nc, ident)
    ones = consts.tile([128, 2, 16], FP8, tag="ones")
    nc.vector.memset(ones, WS)
    eps_t = consts.tile([128, 1], F32, tag="eps")
    nc.vector.memset(eps_t, EPS * WS * WS)

    bq_t = consts.tile([128, H], F32, tag="bq")
    bk_t = consts.tile([128, 4], F32, tag="bk")
    nc.sync.dma_start(out=bq_t, in_=bq.rearrange("(h p) -> p h", p=128))
    nc.sync.dma_start(out=bk_t, in_=bk.rearrange("(h p) -> p h", p=128))

    def bcast128(name, src, n):
        t = consts.tile([128, n], F32, tag=name, name=name)
        src_b = bass.AP(tensor=src.tensor, offset=src.offset,
                        ap=[[0, 128]] + src.ap)
        nc.sync.dma_start(out=t, in_=src_b)
        return t

    bv_bc = bcast128("bv_bc", bv, 512)

    # late-phase tensors (prefetched mid-attention)
    wo_t = w_pool.tile([128, DC, D], FP8, tag="wo")
    xq_t = xq_pool.tile([128, QC, D], BF16, tag="xqs")
    ctx_all = w_pool.tile([128, H, TQ], FP8, tag="ctx_all")

    kv = {}

    def load_kv(h):
        ktr = krem_pool.tile([128, T], BF16, tag="ktr", name=f"ktr{h}")
        nc.sync.dma_start(out=ktr, in_=ap['k_all'][h])
        vr = vrem_pool.tile([128, KC, 128], FP8, tag="vr", name=f"vr{h}")
        nc.sync.dma_start(out=vr, in_=ap['v_all'][h])
        kv[h] = (ktr, vr)

    with contextlib.ExitStack() as es2:
        proj_ps = es2.enter_context(tc.tile_pool(name="proj_ps", bufs=4,
                                                 space="PSUM"))
        kt_pool = es2.enter_context(tc.tile_pool(name="ktl", bufs=1))
        v_pool = es2.enter_context(tc.tile_pool(name="vl", bufs=1))

        # ---- K projection: 4 local heads over all T tokens --------------
        kt_loc = [kt_pool.tile([128, T], BF16, tag=f"ktl{j}", name=f"ktl{j}")
                  for j in range(4)]
        k_send = dram.tile([4, 128, T], BF16, tag="k_send")
        k_all = dram.tile([H, 128, T], BF16, tag="k_all")
        v_send = dram.tile([4, 128, KC, 128], FP8, tag="v_send")
        v_all = dram.tile([H, 128, KC, 128], FP8, tag="v_all")
        ap['k_all'], ap['v_all'] = k_all, v_all

        for j in range(4):
            jsl = slice(j * 128, (j + 1) * 128)
            for nt in range(T // 512):
                nsl = slice(nt * 512, (nt + 1) * 512)
                pp = proj_ps.tile([128, 512], F32, tag="ps", name="pp")
                for i in range(DC // 2):
                    nc.tensor.matmul(pp, wk_t[:, 2 * i:2 * i + 2, jsl],
                                     xt[:, 2 * i:2 * i + 2, nsl],
                                     start=(i == 0), stop=(i == DC // 2 - 1),
                                     perf_mode=DR)
                nc.vector.tensor_scalar(out=kt_loc[j][:, nsl], in0=pp,
                                        scalar1=bk_t[:, j:j + 1],
                                        scalar2=None, op0=ALU.add)
            nc.sync.dma_start(out=k_send[j], in_=kt_loc[j])
        nc.gpsimd.collective_compute(
            "AllGather", mybir.AluOpType.bypass,
            ins=[k_send.opt()], outs=[k_all.opt()],
            replica_groups=[[0, 1], [2, 3], [4, 5], [6, 7]])

        # ---- V projection (x stationary): v_loc4[tok, kc, 4*dh] ---------
        v_loc4 = v_pool.tile([128, KC, 512], FP8, tag="v4")
        for kc in range(KC):
            ksl = slice(kc * 128, (kc + 1) * 128)
            vp = proj_ps.tile([128, 512], F32, tag="ps", name="vp")
            for i in range(DC // 2):
                nc.tensor.matmul(vp, xt[:, 2 * i:2 * i + 2, ksl],
                                 wv_t[:, 2 * i:2 * i + 2, :],
                                 start=(i == 0), stop=(i == DC // 2 - 1),
                                 perf_mode=DR)
            nc.vector.tensor_tensor(out=v_loc4[:, kc, :], in0=vp,
                                    in1=bv_bc, op=ALU.add)
        for j in range(4):
            jsl = slice(j * 128, (j + 1) * 128)
            nc.sync.dma_start(out=v_send[j], in_=v_loc4[:, :, jsl])
        nc.gpsimd.collective_compute(
            "AllGather", mybir.AluOpType.bypass,
            ins=[v_send.opt()], outs=[v_all.opt()],
            replica_groups=[[0, 1], [2, 3], [4, 5], [6, 7]])

        # head 0/1 K/V fetches queue behind the gathers
        load_kv(0)
        load_kv(1)

        # ---- Q projection: all 8 heads ----------------------------------
        qt = {}
        for h in range(H):
            hsl = slice(h * 128, (h + 1) * 128)
            qh = qt_pool.tile([128, TQ], BF16, tag=f"qt{h}", name=f"qt{h}")
            for nt in range(TQ // 512):
                nsl = slice(nt * 512, (nt + 1) * 512)
                qp = proj_ps.tile([128, 512], F32, tag="ps", name="qp")
                for i in range(DC // 2):
                    nc.tensor.matmul(qp, wq_t[:, 2 * i:2 * i + 2, hsl],
                                     xt[:, 2 * i:2 * i + 2, nsl],
                                     start=(i == 0), stop=(i == DC // 2 - 1),
                                     perf_mode=DR)
                nc.vector.tensor_scalar(out=qh[:, nsl], in0=qp,
                                        scalar1=bq_t[:, h:h + 1],
                                        scalar2=None, op0=ALU.add)
            qt[h] = qh

    # ---- attention ------------------------------------------------------
    with contextlib.ExitStack() as es3:
        s_psum = es3.enter_context(tc.tile_pool(name="s_ps", bufs=2,
                                                space="PSUM"))
        ctx_psum = es3.enter_context(tc.tile_pool(name="ctx_ps", bufs=1,
                                                  space="PSUM"))
        sum_psum = es3.enter_context(tc.tile_pool(name="sum_ps", bufs=1,
                                                  space="PSUM"))

        for h in range(H):
            if h + 2 < H:
                load_kv(h + 2)
            kt_h, v_h = kv.pop(h)
            qt_h = qt[h]
            if h == 1:
                nc.sync.dma_start(out=wo_t, in_=Wo)
            if h == 2:
                for qc2 in range(QC):
                    nc.sync.dma_start(out=xq_t[:, qc2, :], in_=xq8[qc2])
                gb = [bcast128("gamma_b", gamma, D),
                      bcast128("beta_b", beta, D)] if apply_gb else None

            ctx_ps = ctx_psum.tile([128, TQ], F32, tag="ctx_ps")
            sum_ps = sum_psum.tile([1, TQ], F32, tag="sum_ps")

            def scores_exp(pair):
                pt = pt_pool.tile([128, 2, TQ], FP8, tag="pt", name="pt")
                for u in range(2):
                    kc = 2 * pair + u
                    ksl = slice(kc * 128, (kc + 1) * 128)
                    s_ps = s_psum.tile([128, TQ], F32, tag="s", name="s_ps")
                    for nq in range(TQ // 512):
                        nsl = slice(nq * 512, (nq + 1) * 512)
                        nc.tensor.matmul(s_ps[:, nsl], kt_h[:, ksl],
                                         qt_h[:, nsl], start=True, stop=True)
                    nc.scalar.activation(out=pt[:, u, :], in_=s_ps,
                                         func=AF.Exp, scale=SC_EXP)
                return pt

            pt_cur = scores_exp(0)
            for pair in range(KC // 2):
                pt_next = scores_exp(pair + 1) if pair + 1 < KC // 2 else None
                first, last = (pair == 0), (pair == KC // 2 - 1)
                for nq in range(TQ // 512):
                    nsl = slice(nq * 512, (nq + 1) * 512)
                    nc.tensor.matmul(ctx_ps[:, nsl],
                                     v_h[:, 2 * pair:2 * pair + 2, :],
                                     pt_cur[:, :, nsl],
                                     start=first, stop=last, perf_mode=DR)
                for nq in range(TQ // 512):
                    nsl = slice(nq * 512, (nq + 1) * 512)
                    nc.tensor.matmul(sum_ps[:, nsl], ones[:, :, 0:1],
                                     pt_cur[:, :, nsl],
                                     start=first, stop=last, perf_mode=DR)
                pt_cur = pt_next

            # drain PSUM fast, normalize off the critical path
            ctx_bf = cb_pool.tile([128, TQ], BF16, tag="cbf")
            nc.vector.tensor_copy(out=ctx_bf, in_=ctx_ps)
            ssb = sums_pool.tile([1, TQ], F32, tag="ssb")
            nc.vector.tensor_copy(out=ssb, in_=sum_ps)
            rsum = sums_pool.tile([1, TQ], F32, tag="rsum")
            nc.vector.reciprocal_approx_fast(out=rsum, in_=ssb)
            rsum_b = sums_pool.tile([128, TQ], F32, tag="rsum_b")
            nc.gpsimd.partition_broadcast(rsum_b, rsum, channels=128)
            nc.vector.tensor_tensor(out=ctx_all[:, h, :], in0=ctx_bf,
                                    in1=rsum_b, op=ALU.mult)

    # ---- out-projection + residual + LayerNorm --------------------------
    with tc.tile_pool(name="y_ps", bufs=2, space="PSUM") as y_psum, \
            tc.tile_pool(name="ln", bufs=4) as ln_pool:
        for qc in range(QC):
            qsl = slice(qc * 128, (qc + 1) * 128)
            y_ps = y_psum.tile([128, D], F32, tag="y_ps")
            for no in range(D // 512):
                nsl = slice(no * 512, (no + 1) * 512)
                nc.tensor.matmul(y_ps[:, nsl], ident, xq_t[:, qc, nsl],
                                 start=True, stop=False)
                for i in range(H // 2):
                    nc.tensor.matmul(y_ps[:, nsl],
                                     ctx_all[:, 2 * i:2 * i + 2, qsl],
                                     wo_t[:, 2 * i:2 * i + 2, nsl],
                                     start=False, stop=(i == H // 2 - 1),
                                     perf_mode=DR)

            stats = ln_pool.tile([128, 2, 6], F32, tag="stats")
            nc.vector.bn_stats(out=stats[:, 0, :], in_=y_ps[:, 0:512])
            nc.vector.bn_stats(out=stats[:, 1, :], in_=y_ps[:, 512:1024])
            mv = ln_pool.tile([128, 2], F32, tag="mv")
            nc.vector.bn_aggr(out=mv, in_=stats)
            std = ln_pool.tile([128, 1], F32, tag="std")
            nc.scalar.activation(out=std, in_=mv[:, 1:2], func=AF.Sqrt,
                                 bias=eps_t)
            rstd = ln_pool.tile([128, 1], F32, tag="rstd")
            nc.vector.reciprocal(out=rstd, in_=std)
            y2 = y2_pool.tile([128, D], F32, tag="y2")
            nc.vector.tensor_scalar(out=y2, in0=y_ps, scalar1=mv[:, 0:1],
                                    scalar2=rstd, op0=ALU.subtract,
                                    op1=ALU.mult)
            if apply_gb:
                nc.vector.tensor_mul(out=y2, in0=y2, in1=gb[0])
                nc.vector.tensor_add(out=y2, in0=y2, in1=gb[1])
            nc.sync.dma_start(out=y[qsl, :], in_=y2)


def build(apply_gb=True):
    nc = bacc.Bacc("TRN2", target_bir_lowering=False, debug=False,
                   enable_asserts=False, num_devices=N_CORES)
    ap = {}
    ap['xt8'] = nc.dram_tensor("xt8", [128, DC, T], FP8,
                               kind="ExternalInput").ap()
    ap['xq8'] = nc.dram_tensor("xq8", [QC, 128, D], BF16,
                               kind="ExternalInput").ap()
    ap['Wq'] = nc.dram_tensor("Wq", [128, DC, D], FP8,
                              kind="ExternalInput").ap()
    ap['bq'] = nc.dram_tensor("bq", [D], F32, kind="ExternalInput").ap()
    ap['Wk'] = nc.dram_tensor("Wk", [128, DC, 512], FP8,
                              kind="ExternalInput").ap()
    ap['bk'] = nc.dram_tensor("bk", [512], F32, kind="ExternalInput").ap()
    ap['Wv'] = nc.dram_tensor("Wv", [128, DC, 512], FP8,
                              kind="ExternalInput").ap()
    ap['bv'] = nc.dram_tensor("bv", [512], F32, kind="ExternalInput").ap()
    ap['Wo'] = nc.dram_tensor("Wo", [128, DC, D], FP8,
                              kind="ExternalInput").ap()
    ap['gamma'] = nc.dram_tensor("gamma", [D], F32, kind="ExternalInput").ap()
    ap['beta'] = nc.dram_tensor("beta", [D], F32, kind="ExternalInput").ap()
    ap['y'] = nc.dram_tensor("y", [TQ, D], F32, kind="ExternalOutput").ap()

    with tile.TileContext(nc) as tc, contextlib.ExitStack() as es:
        _body(nc, tc, ap, es, apply_gb)
    nc.compile()
    return nc


def _pack_rows(w):
    """[D, N] -> [128, DC, N] with rows (c*128+p) -> [p, c]."""
    n = w.shape[1]
    return np.ascontiguousarray(
        w.reshape(DC, 128, n).transpose(1, 0, 2))


def make_in_maps(inputs):
    """Per-core input maps; x token-rotated so q tokens come first."""
    f32 = {k: np.ascontiguousarray(np.asarray(v, dtype=np.float32))
           for k, v in inputs.items()}
    shared = {
        'Wq': _pack_rows(WS * f32['Wq']).astype(E4),
        'Wo': _pack_rows(WS * f32['Wo']).astype(E4),
        'bq': WS * f32['bq'],
        'gamma': f32['gamma'], 'beta': f32['beta'],
    }
    wk8 = WS * f32['Wk']
    wv8 = WS * f32['Wv']
    x = f32['x']
    in_maps = []
    for core in range(N_CORES):
        b, gg = divmod(core, 2)
        own = slice(512 * gg, 512 * (gg + 1))
        xr = np.roll(x[b], -TQ * gg, axis=0)
        xq8 = (WS * (xr[:TQ] + f32['bo'])).astype(BF)
        in_maps.append({
            'xt8': _pack_rows(xr.T).astype(E4),
            'xq8': np.ascontiguousarray(xq8.reshape(QC, 128, D)),
            'Wk': _pack_rows(wk8[:, own]).astype(E4),
            'bk': WS * f32['bk'][own],
            'Wv': _pack_rows(wv8[:, own]).astype(E4),
            'bv': WS * f32['bv'][own],
            **shared})
    return in_maps


_NC = {}


def kernel(**inputs):
    apply_gb = not (np.all(np.asarray(inputs['gamma']) == 1.0)
                    and np.all(np.asarray(inputs['beta']) == 0.0))
    in_maps = make_in_maps(inputs)
    if apply_gb not in _NC:
        _NC[apply_gb] = build(apply_gb)
    res = bass_utils.run_bass_kernel_spmd(_NC[apply_gb], in_maps,
                                          core_ids=list(range(N_CORES)))
    out = np.empty((B, T, D), dtype=np.float32)
    for core in range(N_CORES):
        b, gg = divmod(core, 2)
        out[b, TQ * gg:TQ * (gg + 1)] = res.results[core]['y']
    return out


# revision 13
# speedup vs baseline: 1.5324x; 1.0593x over previous
"""Multi-head attention block (QKV proj + softmax attention + out-proj +
residual + LayerNorm) on 8 TRN2 NeuronCores.

Sharding: core = (batch b, token-half g). Each core computes attention for
its 1024 query tokens over all 8 heads. K/V for the core's 4 local heads
are computed over the full 2048 tokens and exchanged with the pair partner
via AllGather; the gather latency hides under the V/Q projections.

Precision: weights are host-scaled by 8 and cast to fp8e4 (dodges fp8
subnormals; compensated exactly: exp scale /64 for Q*K, ones=8 for the
softmax denominator, LayerNorm scale-invariance with eps*64 for the
residual path). Matmuls with contraction >=256 run fp8 DoubleRow (2
contraction rows per pass); scores run bf16 (contraction = head dim 128).
The V projection runs with x as the stationary operand, producing
v[token, dh] directly (no PE transposes). The residual is injected into
the out-projection PSUM via an identity matmul. Accumulation is f32 in
PSUM; softmax statistics and LayerNorm are f32.
"""

import contextlib
import sys

if '/opt/trn_rl_repo' not in sys.path:
    sys.path.insert(0, '/opt/trn_rl_repo')

import ml_dtypes
import numpy as np

import concourse.bacc as bacc
import concourse.bass as bass
import concourse.bass_utils as bass_utils
import concourse.tile as tile
from concourse import mybir
from concourse.masks import make_identity

B, T, D, H = 4, 2048, 1024, 8
DH = 128            # head dim
TQ = T // 2         # query tokens per core
N_CORES = 8
DC = D // 128       # d-chunks of 128
KC = T // 128       # k-token chunks of 128
QC = TQ // 128      # q-token chunks of 128
EPS = 1e-5
WS = 8.0            # host-side weight scale (keeps fp8 weights normal)
SC_EXP = 1.0 / (float(np.sqrt(DH)) * WS * WS)
F32 = mybir.dt.float32
BF16 = mybir.dt.bfloat16
FP8 = mybir.dt.float8e4
AF = mybir.ActivationFunctionType
ALU = mybir.AluOpType
DR = mybir.MatmulPerfMode.DoubleRow
BF = ml_dtypes.bfloat16
E4 = ml_dtypes.float8_e4m3


def _body(nc, tc, ap, es, apply_gb):
    xt8, xq8, Wq, bq, Wk, bk, Wv, bv, Wo, gamma, beta, y = (
        ap['xt8'], ap['xq8'], ap['Wq'], ap['bq'], ap['Wk'], ap['bk'],
        ap['Wv'], ap['bv'], ap['Wo'], ap['gamma'], ap['beta'], ap['y'])

    consts = es.enter_context(tc.tile_pool(name="consts", bufs=1))
    w_pool = es.enter_context(tc.tile_pool(name="w", bufs=1))
    krem_pool = es.enter_context(tc.tile_pool(name="krem", bufs=3))
    vrem_pool = es.enter_context(tc.tile_pool(name="vrem", bufs=3))
    qt_pool = es.enter_context(tc.tile_pool(name="qt", bufs=1))
    pt_pool = es.enter_context(tc.tile_pool(name="pt", bufs=2))
    cb_pool = es.enter_context(tc.tile_pool(name="cb", bufs=2))
    sums_pool = es.enter_context(tc.tile_pool(name="sums", bufs=2))
    y2_pool = es.enter_context(tc.tile_pool(name="y2", bufs=2))
    xq_pool = es.enter_context(tc.tile_pool(name="xq", bufs=1))
    dram = es.enter_context(tc.tile_pool(name="dram", bufs=1, space="DRAM"))

    # ---- weight / x loads (issue order = DMA priority) -------------------
    wk_t = w_pool.tile([128, DC, 512], FP8, tag="wk")
    nc.sync.dma_start(out=wk_t, in_=Wk)
    xt = w_pool.tile([128, DC, T], FP8, tag="xt")
    for tb in range(4):
        tsl = slice(tb * 512, (tb + 1) * 512)
        nc.sync.dma_start(out=xt[:, :, tsl], in_=xt8[:, :, tsl])
    wv_t = w_pool.tile([128, DC, 512], FP8, tag="wv")
    nc.sync.dma_start(out=wv_t, in_=Wv)
    wq_t = w_pool.tile([128, DC, D], FP8, tag="wq")
    nc.sync.dma_start(out=wq_t, in_=Wq)

    ident = consts.tile([128, 128], BF16, tag="ident")
    make_identity(nc, ident)
    ones = consts.tile([128, 2, 16], FP8, tag="ones")
    nc.vector.memset(ones, WS)
    eps_t = consts.tile([128, 1], F32, tag="eps")
    nc.vector.memset(eps_t, EPS * WS * WS)

    bq_t = consts.tile([128, H], F32, tag="bq")
    bk_t = consts.tile([128, 4], F32, tag="bk")
    nc.sync.dma_start(out=bq_t, in_=bq.rearrange("(h p) -> p h", p=128))
    nc.sync.dma_start(out=bk_t, in_=bk.rearrange("(h p) -> p h", p=128))

    def bcast128(name, src, n):
        t = consts.tile([128, n], F32, tag=name, name=name)
        src_b = bass.AP(tensor=src.tensor, offset=src.offset,
                        ap=[[0, 128]] + src.ap)
        nc.sync.dma_start(out=t, in_=src_b)
        return t

    bv_bc = bcast128("bv_bc", bv, 512)

    # late-phase tensors (prefetched mid-attention)
    wo_t = w_pool.tile([128, DC, D], FP8, tag="wo")
    xq_t = xq_pool.tile([128, QC, D], BF16, tag="xqs")
    ctx_all = w_pool.tile([128, QC, H, 128], FP8, tag="ctx_all")

    kv = {}

    def load_kv(h):
        ktr = krem_pool.tile([128, T], FP8, tag="ktr", name=f"ktr{h}")
        nc.sync.dma_start(out=ktr, in_=ap['kv_all'][h][:, 0:T])
        vr = vrem_pool.tile([128, KC, 128], FP8, tag="vr", name=f"vr{h}")
        nc.sync.dma_start(out=vr.rearrange("p a b -> p (a b)"),
                          in_=ap['kv_all'][h][:, T:2 * T])
        kv[h] = (ktr, vr)

    with contextlib.ExitStack() as es2:
        proj_ps = es2.enter_context(tc.tile_pool(name="proj_ps", bufs=4,
                                                 space="PSUM"))
        kt_pool = es2.enter_context(tc.tile_pool(name="ktl", bufs=1))
        v_pool = es2.enter_context(tc.tile_pool(name="vl", bufs=1))

        # ---- K projection: 4 local heads over all T tokens --------------
        kt_loc = [kt_pool.tile([128, T], FP8, tag=f"ktl{j}", name=f"ktl{j}")
                  for j in range(4)]
        kv_send = dram.tile([4, 128, 2 * T], FP8, tag="kv_send")
        kv_all = dram.tile([H, 128, 2 * T], FP8, tag="kv_all")
        ap['kv_all'] = kv_all

        for j in range(4):
            jsl = slice(j * 128, (j + 1) * 128)
            for nt in range(T // 512):
                nsl = slice(nt * 512, (nt + 1) * 512)
                pp = proj_ps.tile([128, 512], F32, tag="ps", name="pp")
                for i in range(DC // 2):
                    nc.tensor.matmul(pp, wk_t[:, 2 * i:2 * i + 2, jsl],
                                     xt[:, 2 * i:2 * i + 2, nsl],
                                     start=(i == 0), stop=(i == DC // 2 - 1),
                                     perf_mode=DR)
                nc.vector.tensor_scalar(out=kt_loc[j][:, nsl], in0=pp,
                                        scalar1=bk_t[:, j:j + 1],
                                        scalar2=None, op0=ALU.add)
            nc.sync.dma_start(out=kv_send[j][:, 0:T], in_=kt_loc[j])

        # ---- V projection (x stationary): v_loc4[tok, kc, 4*dh] ---------
        v_loc4 = v_pool.tile([128, KC, 512], FP8, tag="v4")
        for kc in range(KC):
            ksl = slice(kc * 128, (kc + 1) * 128)
            vp = proj_ps.tile([128, 512], F32, tag="ps", name="vp")
            for i in range(DC // 2):
                nc.tensor.matmul(vp, xt[:, 2 * i:2 * i + 2, ksl],
                                 wv_t[:, 2 * i:2 * i + 2, :],
                                 start=(i == 0), stop=(i == DC // 2 - 1),
                                 perf_mode=DR)
            nc.vector.tensor_tensor(out=v_loc4[:, kc, :], in0=vp,
                                    in1=bv_bc, op=ALU.add)
        for j in range(4):
            jsl = slice(j * 128, (j + 1) * 128)
            nc.sync.dma_start(
                out=kv_send[j][:, T:2 * T].rearrange("p (a b) -> p a b",
                                                     b=128),
                in_=v_loc4[:, :, jsl])
        nc.gpsimd.collective_compute(
            "AllGather", mybir.AluOpType.bypass,
            ins=[kv_send.opt()], outs=[kv_all.opt()],
            replica_groups=[[0, 1], [2, 3], [4, 5], [6, 7]])

        # head 0/1 K/V fetches queue behind the gather
        load_kv(0)
        load_kv(1)

        # ---- Q projection: all 8 heads ----------------------------------
        qt = {}
        for h in range(H):
            hsl = slice(h * 128, (h + 1) * 128)
            qh = qt_pool.tile([128, TQ], BF16, tag=f"qt{h}", name=f"qt{h}")
            for nt in range(TQ // 512):
                nsl = slice(nt * 512, (nt + 1) * 512)
                qp = proj_ps.tile([128, 512], F32, tag="ps", name="qp")
                for i in range(DC // 2):
                    nc.tensor.matmul(qp, wq_t[:, 2 * i:2 * i + 2, hsl],
                                     xt[:, 2 * i:2 * i + 2, nsl],
                                     start=(i == 0), stop=(i == DC // 2 - 1),
                                     perf_mode=DR)
                nc.vector.tensor_scalar(out=qh[:, nsl], in0=qp,
                                        scalar1=bq_t[:, h:h + 1],
                                        scalar2=None, op0=ALU.add)
            qt[h] = qh

    # ---- attention ------------------------------------------------------
    with contextlib.ExitStack() as es3:
        s_psum = es3.enter_context(tc.tile_pool(name="s_ps", bufs=2,
                                                space="PSUM"))
        ctx_psum = es3.enter_context(tc.tile_pool(name="ctx_ps", bufs=1,
                                                  space="PSUM"))
        sum_psum = es3.enter_context(tc.tile_pool(name="sum_ps", bufs=1,
                                                  space="PSUM"))

        for h in range(H):
            if h + 2 < H:
                load_kv(h + 2)
            kt_h, v_h = kv.pop(h)
            qt_h = qt[h]
            if h == 1:
                nc.sync.dma_start(out=wo_t, in_=Wo)
            if h == 2:
                for qc2 in range(QC):
                    nc.sync.dma_start(out=xq_t[:, qc2, :], in_=xq8[qc2])
                gb = [bcast128("gamma_b", gamma, D),
                      bcast128("beta_b", beta, D)] if apply_gb else None

            ctx_ps = ctx_psum.tile([128, TQ], F32, tag="ctx_ps")
            sum_ps = sum_psum.tile([1, TQ], F32, tag="sum_ps")

            def scores_exp(pair):
                pt = pt_pool.tile([128, 2, TQ], FP8, tag="pt", name="pt")
                for u in range(2):
                    kc = 2 * pair + u
                    ksl = slice(kc * 128, (kc + 1) * 128)
                    s_ps = s_psum.tile([128, TQ], F32, tag="s", name="s_ps")
                    for nq in range(TQ // 512):
                        nsl = slice(nq * 512, (nq + 1) * 512)
                        nc.tensor.matmul(s_ps[:, nsl], kt_h[:, ksl],
                                         qt_h[:, nsl], start=True, stop=True)
                    nc.scalar.activation(out=pt[:, u, :], in_=s_ps,
                                         func=AF.Exp, scale=SC_EXP)
                return pt

            pt_cur = scores_exp(0)
            for pair in range(KC // 2):
                pt_next = scores_exp(pair + 1) if pair + 1 < KC // 2 else None
                first, last = (pair == 0), (pair == KC // 2 - 1)
                for nq in range(TQ // 512):
                    nsl = slice(nq * 512, (nq + 1) * 512)
                    nc.tensor.matmul(ctx_ps[:, nsl],
                                     v_h[:, 2 * pair:2 * pair + 2, :],
                                     pt_cur[:, :, nsl],
                                     start=first, stop=last, perf_mode=DR)
                for nq in range(TQ // 512):
                    nsl = slice(nq * 512, (nq + 1) * 512)
                    nc.tensor.matmul(sum_ps[:, nsl], ones[:, :, 0:1],
                                     pt_cur[:, :, nsl],
                                     start=first, stop=last, perf_mode=DR)
                pt_cur = pt_next

            # drain PSUM fast, normalize off the critical path
            ctx_bf = cb_pool.tile([128, TQ], BF16, tag="cbf")
            nc.vector.tensor_copy(out=ctx_bf, in_=ctx_ps)
            rsum = sums_pool.tile([1, TQ], F32, tag="rsum")
            nc.vector.reciprocal_approx_fast(out=rsum, in_=sum_ps)
            rsum_b = sums_pool.tile([128, TQ], F32, tag="rsum_b")
            nc.gpsimd.partition_broadcast(rsum_b, rsum, channels=128)
            nc.vector.tensor_tensor(
                out=ctx_all[:, :, h, :],
                in0=ctx_bf.rearrange("p (a c) -> p a c", c=128),
                in1=rsum_b.rearrange("p (a c) -> p a c", c=128),
                op=ALU.mult)

    # ---- out-projection + residual + LayerNorm --------------------------
    with tc.tile_pool(name="y_ps", bufs=2, space="PSUM") as y_psum, \
            tc.tile_pool(name="ln", bufs=4) as ln_pool:
        for qc in range(QC):
            qsl = slice(qc * 128, (qc + 1) * 128)
            y_ps = y_psum.tile([128, D], F32, tag="y_ps")
            for no in range(D // 512):
                nsl = slice(no * 512, (no + 1) * 512)
                nc.tensor.matmul(y_ps[:, nsl], ident, xq_t[:, qc, nsl],
                                 start=True, stop=False)
                for i in range(H // 2):
                    nc.tensor.matmul(y_ps[:, nsl],
                                     ctx_all[:, qc, 2 * i:2 * i + 2, :],
                                     wo_t[:, 2 * i:2 * i + 2, nsl],
                                     start=False, stop=(i == H // 2 - 1),
                                     perf_mode=DR)

            stats = ln_pool.tile([128, 2, 6], F32, tag="stats")
            nc.vector.bn_stats(out=stats[:, 0, :], in_=y_ps[:, 0:512])
            nc.vector.bn_stats(out=stats[:, 1, :], in_=y_ps[:, 512:1024])
            mv = ln_pool.tile([128, 2], F32, tag="mv")
            nc.vector.bn_aggr(out=mv, in_=stats)
            std = ln_pool.tile([128, 1], F32, tag="std")
            nc.scalar.activation(out=std, in_=mv[:, 1:2], func=AF.Sqrt,
                                 bias=eps_t)
            rstd = ln_pool.tile([128, 1], F32, tag="rstd")
            nc.vector.reciprocal(out=rstd, in_=std)
            y2 = y2_pool.tile([128, D], F32, tag="y2")
            nc.vector.tensor_scalar(out=y2, in0=y_ps, scalar1=mv[:, 0:1],
                                    scalar2=rstd, op0=ALU.subtract,
                                    op1=ALU.mult)
            if apply_gb:
                nc.vector.tensor_mul(out=y2, in0=y2, in1=gb[0])
                nc.vector.tensor_add(out=y2, in0=y2, in1=gb[1])
            nc.sync.dma_start(out=y[qsl, :], in_=y2)


def build(apply_gb=True):
    nc = bacc.Bacc("TRN2", target_bir_lowering=False, debug=False,
                   enable_asserts=False, num_devices=N_CORES)
    ap = {}
    ap['xt8'] = nc.dram_tensor("xt8", [128, DC, T], FP8,
                               kind="ExternalInput").ap()
    ap['xq8'] = nc.dram_tensor("xq8", [QC, 128, D], BF16,
                               kind="ExternalInput").ap()
    ap['Wq'] = nc.dram_tensor("Wq", [128, DC, D], FP8,
                              kind="ExternalInput").ap()
    ap['bq'] = nc.dram_tensor("bq", [D], F32, kind="ExternalInput").ap()
    ap['Wk'] = nc.dram_tensor("Wk", [128, DC, 512], FP8,
                              kind="ExternalInput").ap()
    ap['bk'] = nc.dram_tensor("bk", [512], F32, kind="ExternalInput").ap()
    ap['Wv'] = nc.dram_tensor("Wv", [128, DC, 512], FP8,
                              kind="ExternalInput").ap()
    ap['bv'] = nc.dram_tensor("bv", [512], F32, kind="ExternalInput").ap()
    ap['Wo'] = nc.dram_tensor("Wo", [128, DC, D], FP8,
                              kind="ExternalInput").ap()
    ap['gamma'] = nc.dram_tensor("gamma", [D], F32, kind="ExternalInput").ap()
    ap['beta'] = nc.dram_tensor("beta", [D], F32, kind="ExternalInput").ap()
    ap['y'] = nc.dram_tensor("y", [TQ, D], F32, kind="ExternalOutput").ap()

    with tile.TileContext(nc) as tc, contextlib.ExitStack() as es:
        _body(nc, tc, ap, es, apply_gb)
    nc.compile()
    return nc


def _pack_rows(w):
    """[D, N] -> [128, DC, N] with rows (c*128+p) -> [p, c]."""
    n = w.shape[1]
    return np.ascontiguousarray(
        w.reshape(DC, 128, n).transpose(1, 0, 2))


def make_in_maps(inputs):
    """Per-core input maps; x token-rotated so q tokens come first."""
    f32 = {k: np.ascontiguousarray(np.asarray(v, dtype=np.float32))
           for k, v in inputs.items()}
    shared = {
        'Wq': _pack_rows(WS * f32['Wq']).astype(E4),
        'Wo': _pack_rows(WS * f32['Wo']).astype(E4),
        'bq': WS * f32['bq'],
        'gamma': f32['gamma'], 'beta': f32['beta'],
    }
    wk8 = WS * f32['Wk']
    wv8 = WS * f32['Wv']
    x = f32['x']
    in_maps = []
    for core in range(N_CORES):
        b, gg = divmod(core, 2)
        own = slice(512 * gg, 512 * (gg + 1))
        xr = np.roll(x[b], -TQ * gg, axis=0)
        xq8 = (WS * (xr[:TQ] + f32['bo'])).astype(BF)
        in_maps.append({
            'xt8': _pack_rows(xr.T).astype(E4),
            'xq8': np.ascontiguousarray(xq8.reshape(QC, 128, D)),
            'Wk': _pack_rows(wk8[:, own]).astype(E4),
            'bk': WS * f32['bk'][own],
            'Wv': _pack_rows(wv8[:, own]).astype(E4),
            'bv': WS * f32['bv'][own],
            **shared})
    return in_maps


_NC = {}


def kernel(**inputs):
    apply_gb = not (np.all(np.asarray(inputs['gamma']) == 1.0)
                    and np.all(np.asarray(inputs['beta']) == 0.0))
    in_maps = make_in_maps(inputs)
    if apply_gb not in _NC:
        _NC[apply_gb] = build(apply_gb)
    res = bass_utils.run_bass_kernel_spmd(_NC[apply_gb], in_maps,
                                          core_ids=list(range(N_CORES)))
    out = np.empty((B, T, D), dtype=np.float32)
    for core in range(N_CORES):
        b, gg = divmod(core, 2)
        out[b, TQ * gg:TQ * (gg + 1)] = res.results[core]['y']
    return out


# revision 20
# speedup vs baseline: 1.5879x; 1.0362x over previous
"""Multi-head attention block (QKV proj + softmax attention + out-proj +
residual + LayerNorm) on 8 TRN2 NeuronCores.

Sharding: core = (batch b, token-half g). Each core computes attention for
its 1024 query tokens over all 8 heads. K/V for the core's 4 local heads
are computed over the full 2048 tokens and exchanged with the pair partner
via AllGather; the gather latency hides under the V/Q projections.

Precision: weights are host-scaled by 8 and cast to fp8e4 (dodges fp8
subnormals; compensated exactly: exp scale /64 for Q*K, ones=8 for the
softmax denominator, LayerNorm scale-invariance with eps*64 for the
residual path). Matmuls with contraction >=256 run fp8 DoubleRow (2
contraction rows per pass); scores run bf16 (contraction = head dim 128).
The V projection runs with x as the stationary operand, producing
v[token, dh] directly (no PE transposes). The residual is injected into
the out-projection PSUM via an identity matmul. Accumulation is f32 in
PSUM; softmax statistics and LayerNorm are f32.
"""

import contextlib
import sys

if '/opt/trn_rl_repo' not in sys.path:
    sys.path.insert(0, '/opt/trn_rl_repo')

import ml_dtypes
import numpy as np

import concourse.bacc as bacc
import concourse.bass as bass
import concourse.bass_utils as bass_utils
import concourse.tile as tile
from concourse import mybir
from concourse.masks import make_identity

B, T, D, H = 4, 2048, 1024, 8
DH = 128            # head dim
TQ = T // 2         # query tokens per core
N_CORES = 8
DC = D // 128       # d-chunks of 128
KC = T // 128       # k-token chunks of 128
QC = TQ // 128      # q-token chunks of 128
EPS = 1e-5
WS = 8.0            # host-side weight scale (keeps fp8 weights normal)
SC_EXP = 1.0 / (float(np.sqrt(DH)) * WS * WS)
F32 = mybir.dt.float32
BF16 = mybir.dt.bfloat16
FP8 = mybir.dt.float8e4
AF = mybir.ActivationFunctionType
ALU = mybir.AluOpType
DR = mybir.MatmulPerfMode.DoubleRow
BF = ml_dtypes.bfloat16
E4 = ml_dtypes.float8_e4m3


def _body(nc, tc, ap, es, apply_gb):
    xt8, xq8, Wq, bq, Wk, bk, Wv, bv, Wo, gamma, beta, y = (
        ap['xt8'], ap['xq8'], ap['Wq'], ap['bq'], ap['Wk'], ap['bk'],
        ap['Wv'], ap['bv'], ap['Wo'], ap['gamma'], ap['beta'], ap['y'])

    consts = es.enter_context(tc.tile_pool(name="consts", bufs=1))
    w_pool = es.enter_context(tc.tile_pool(name="w", bufs=1))
    kt_pool = es.enter_context(tc.tile_pool(name="ktl", bufs=1))
    v_pool = es.enter_context(tc.tile_pool(name="vl", bufs=1))
    rem_pool = es.enter_context(tc.tile_pool(name="rem", bufs=1))
    qt_pool = es.enter_context(tc.tile_pool(name="qt", bufs=1))
    pt_pool = es.enter_context(tc.tile_pool(name="pt", bufs=2))
    cb_pool = es.enter_context(tc.tile_pool(name="cb", bufs=2))
    sums_pool = es.enter_context(tc.tile_pool(name="sums", bufs=2))
    y2_pool = es.enter_context(tc.tile_pool(name="y2", bufs=2))
    xq_pool = es.enter_context(tc.tile_pool(name="xq", bufs=1))
    dram = es.enter_context(tc.tile_pool(name="dram", bufs=1, space="DRAM"))

    # ---- weight / x loads (issue order = DMA priority) -------------------
    wk_t = w_pool.tile([128, DC, 512], FP8, tag="wk")
    nc.sync.dma_start(out=wk_t, in_=Wk)
    xt = w_pool.tile([128, DC, T], FP8, tag="xt")
    for tb in range(4):
        tsl = slice(tb * 512, (tb + 1) * 512)
        nc.sync.dma_start(out=xt[:, :, tsl], in_=xt8[:, :, tsl])
    wv_t = w_pool.tile([128, DC, 512], FP8, tag="wv")
    nc.sync.dma_start(out=wv_t, in_=Wv)
    wq_t = w_pool.tile([128, DC, D], FP8, tag="wq")
    nc.sync.dma_start(out=wq_t, in_=Wq)

    ident = consts.tile([128, 128], BF16, tag="ident")
    make_identity(nc, ident)
    ones = consts.tile([128, 2, 16], FP8, tag="ones")
    nc.vector.memset(ones, WS)
    eps_t = consts.tile([128, 1], F32, tag="eps")
    nc.vector.memset(eps_t, EPS * WS * WS)

    bq_t = consts.tile([128, H], F32, tag="bq")
    bk_t = consts.tile([128, 4], F32, tag="bk")
    nc.sync.dma_start(out=bq_t, in_=bq.rearrange("(h p) -> p h", p=128))
    nc.sync.dma_start(out=bk_t, in_=bk.rearrange("(h p) -> p h", p=128))

    def bcast128(name, src, n):
        t = consts.tile([128, n], F32, tag=name, name=name)
        src_b = bass.AP(tensor=src.tensor, offset=src.offset,
                        ap=[[0, 128]] + src.ap)
        nc.sync.dma_start(out=t, in_=src_b)
        return t

    bv_bc = bcast128("bv_bc", bv, 512)

    # late-phase tensors (prefetched mid-attention)
    wo_t = w_pool.tile([128, DC, D], FP8, tag="wo")
    xq_t = xq_pool.tile([128, QC, D], BF16, tag="xqs")
    ctx_all = w_pool.tile([128, QC, H, 128], FP8, tag="ctx_all")

    kt_loc = [kt_pool.tile([128, T], FP8, tag=f"ktl{j}", name=f"ktl{j}")
              for j in range(4)]
    v_loc = [v_pool.tile([128, KC, 128], FP8, tag=f"vl{j}", name=f"vl{j}")
             for j in range(4)]
    kt_rem = [rem_pool.tile([128, T], FP8, tag=f"ktr{j}", name=f"ktr{j}")
              for j in range(4)]
    v_rem = [rem_pool.tile([128, KC, 128], FP8, tag=f"vr{j}", name=f"vr{j}")
             for j in range(4)]
    kv_send = dram.tile([4, 128, 2 * T], FP8, tag="kv_send")
    kv_sum = dram.tile([4, 128, 2 * T], FP8, tag="kv_sum")

    def fetch_remote(j):
        # remote K/V = pair-sum minus local, recovered on the idle GpSimd
        nc.sync.dma_start(out=kt_rem[j], in_=kv_sum[j][:, 0:T])
        nc.sync.dma_start(out=v_rem[j].rearrange("p a b -> p (a b)"),
                          in_=kv_sum[j][:, T:2 * T])
        nc.gpsimd.tensor_tensor(out=kt_rem[j], in0=kt_rem[j],
                                in1=kt_loc[j], op=ALU.subtract)
        nc.gpsimd.tensor_tensor(out=v_rem[j], in0=v_rem[j], in1=v_loc[j],
                                op=ALU.subtract)

    with contextlib.ExitStack() as es2:
        proj_ps = es2.enter_context(tc.tile_pool(name="proj_ps", bufs=4,
                                                 space="PSUM"))

        # ---- K projection: 4 local heads over all T tokens --------------
        for j in range(4):
            jsl = slice(j * 128, (j + 1) * 128)
            for nt in range(T // 512):
                nsl = slice(nt * 512, (nt + 1) * 512)
                pp = proj_ps.tile([128, 512], F32, tag="ps", name="pp")
                for i in range(DC // 2):
                    nc.tensor.matmul(pp, wk_t[:, 2 * i:2 * i + 2, jsl],
                                     xt[:, 2 * i:2 * i + 2, nsl],
                                     start=(i == 0), stop=(i == DC // 2 - 1),
                                     perf_mode=DR)
                nc.vector.tensor_scalar(out=kt_loc[j][:, nsl], in0=pp,
                                        scalar1=bk_t[:, j:j + 1],
                                        scalar2=None, op0=ALU.add)
            nc.sync.dma_start(out=kv_send[j][:, 0:T], in_=kt_loc[j])

        # ---- V projection (x stationary): per-chunk, all 4 local heads --
        for kc in range(KC):
            ksl = slice(kc * 128, (kc + 1) * 128)
            vp = proj_ps.tile([128, 512], F32, tag="ps", name="vp")
            for i in range(DC // 2):
                nc.tensor.matmul(vp, xt[:, 2 * i:2 * i + 2, ksl],
                                 wv_t[:, 2 * i:2 * i + 2, :],
                                 start=(i == 0), stop=(i == DC // 2 - 1),
                                 perf_mode=DR)
            for j in range(4):
                jsl = slice(j * 128, (j + 1) * 128)
                nc.vector.tensor_tensor(out=v_loc[j][:, kc, :],
                                        in0=vp[:, jsl],
                                        in1=bv_bc[:, jsl], op=ALU.add)
        for j in range(4):
            nc.sync.dma_start(
                out=kv_send[j][:, T:2 * T].rearrange("p (a b) -> p a b",
                                                     b=128),
                in_=v_loc[j])
        nc.gpsimd.collective_compute(
            "AllReduce", ALU.add,
            ins=[kv_send.opt()], outs=[kv_sum.opt()],
            replica_groups=[[0, 1], [2, 3], [4, 5], [6, 7]])

        # ---- Q projection: all 8 slots ----------------------------------
        qt = {}
        for h in range(H):
            hsl = slice(h * 128, (h + 1) * 128)
            qh = qt_pool.tile([128, TQ], BF16, tag=f"qt{h}", name=f"qt{h}")
            for nt in range(TQ // 512):
                nsl = slice(nt * 512, (nt + 1) * 512)
                qp = proj_ps.tile([128, 512], F32, tag="ps", name="qp")
                for i in range(DC // 2):
                    nc.tensor.matmul(qp, wq_t[:, 2 * i:2 * i + 2, hsl],
                                     xt[:, 2 * i:2 * i + 2, nsl],
                                     start=(i == 0), stop=(i == DC // 2 - 1),
                                     perf_mode=DR)
                nc.vector.tensor_scalar(out=qh[:, nsl], in0=qp,
                                        scalar1=bq_t[:, h:h + 1],
                                        scalar2=None, op0=ALU.add)
            qt[h] = qh

    # ---- attention ------------------------------------------------------
    with contextlib.ExitStack() as es3:
        s_psum = es3.enter_context(tc.tile_pool(name="s_ps", bufs=2,
                                                space="PSUM"))
        ctx_psum = es3.enter_context(tc.tile_pool(name="ctx_ps", bufs=1,
                                                  space="PSUM"))
        sum_psum = es3.enter_context(tc.tile_pool(name="sum_ps", bufs=1,
                                                  space="PSUM"))

        for h in range(H):
            if h < 4:
                kt_h, v_h = kt_loc[h], v_loc[h]
            else:
                kt_h, v_h = kt_rem[h - 4], v_rem[h - 4]
            qt_h = qt[h]
            if h == 1:
                nc.sync.dma_start(out=wo_t, in_=Wo)
            if h == 2:
                for qc2 in range(QC):
                    nc.sync.dma_start(out=xq_t[:, qc2, :], in_=xq8[qc2])
                gb = [bcast128("gamma_b", gamma, D),
                      bcast128("beta_b", beta, D)] if apply_gb else None

            ctx_ps = ctx_psum.tile([128, TQ], F32, tag="ctx_ps")
            sum_ps = sum_psum.tile([1, TQ], F32, tag="sum_ps")

            def scores_exp(pair):
                pt = pt_pool.tile([128, 2, TQ], FP8, tag="pt", name="pt")
                for u in range(2):
                    kc = 2 * pair + u
                    ksl = slice(kc * 128, (kc + 1) * 128)
                    s_ps = s_psum.tile([128, TQ], F32, tag="s", name="s_ps")
                    for nq in range(TQ // 512):
                        nsl = slice(nq * 512, (nq + 1) * 512)
                        nc.tensor.matmul(s_ps[:, nsl], kt_h[:, ksl],
                                         qt_h[:, nsl], start=True, stop=True)
                    nc.scalar.activation(out=pt[:, u, :], in_=s_ps,
                                         func=AF.Exp, scale=SC_EXP)
                return pt

            pt_cur = scores_exp(0)
            for pair in range(KC // 2):
                pt_next = scores_exp(pair + 1) if pair + 1 < KC // 2 else None
                first, last = (pair == 0), (pair == KC // 2 - 1)
                for nq in range(TQ // 512):
                    nsl = slice(nq * 512, (nq + 1) * 512)
                    nc.tensor.matmul(ctx_ps[:, nsl],
                                     v_h[:, 2 * pair:2 * pair + 2, :],
                                     pt_cur[:, :, nsl],
                                     start=first, stop=last, perf_mode=DR)
                for nq in range(TQ // 512):
                    nsl = slice(nq * 512, (nq + 1) * 512)
                    nc.tensor.matmul(sum_ps[:, nsl], ones[:, :, 0:1],
                                     pt_cur[:, :, nsl],
                                     start=first, stop=last, perf_mode=DR)
                pt_cur = pt_next

            # drain PSUM fast, normalize off the critical path
            ctx_bf = cb_pool.tile([128, TQ], BF16, tag="cbf")
            nc.vector.tensor_copy(out=ctx_bf, in_=ctx_ps)
            rsum = sums_pool.tile([1, TQ], F32, tag="rsum")
            nc.vector.reciprocal_approx_fast(out=rsum, in_=sum_ps)
            rsum_b = sums_pool.tile([128, TQ], F32, tag="rsum_b")
            nc.gpsimd.partition_broadcast(rsum_b, rsum, channels=128)
            nc.vector.tensor_tensor(
                out=ctx_all[:, :, h, :],
                in0=ctx_bf.rearrange("p (a c) -> p a c", c=128),
                in1=rsum_b.rearrange("p (a c) -> p a c", c=128),
                op=ALU.mult)
            if h < 4:
                fetch_remote(h)

    # ---- out-projection + residual + LayerNorm --------------------------
    with tc.tile_pool(name="y_ps", bufs=3, space="PSUM") as y_psum, \
            tc.tile_pool(name="ln", bufs=4) as ln_pool:
        for qc in range(QC):
            qsl = slice(qc * 128, (qc + 1) * 128)
            y_ps = y_psum.tile([128, D], F32, tag="y_ps")
            for no in range(D // 512):
                nsl = slice(no * 512, (no + 1) * 512)
                nc.tensor.matmul(y_ps[:, nsl], ident, xq_t[:, qc, nsl],
                                 start=True, stop=False)
                for i in range(H // 2):
                    nc.tensor.matmul(y_ps[:, nsl],
                                     ctx_all[:, qc, 2 * i:2 * i + 2, :],
                                     wo_t[:, 2 * i:2 * i + 2, nsl],
                                     start=False, stop=(i == H // 2 - 1),
                                     perf_mode=DR)

            stats = ln_pool.tile([128, 2, 6], F32, tag="stats")
            nc.vector.bn_stats(out=stats[:, 0, :], in_=y_ps[:, 0:512])
            nc.vector.bn_stats(out=stats[:, 1, :], in_=y_ps[:, 512:1024])
            mv = ln_pool.tile([128, 2], F32, tag="mv")
            nc.vector.bn_aggr(out=mv, in_=stats)
            std = ln_pool.tile([128, 1], F32, tag="std")
            nc.scalar.activation(out=std, in_=mv[:, 1:2], func=AF.Sqrt,
                                 bias=eps_t)
            rstd = ln_pool.tile([128, 1], F32, tag="rstd")
            nc.vector.reciprocal(out=rstd, in_=std)
            y2 = y2_pool.tile([128, D], F32, tag="y2")
            nc.vector.tensor_scalar(out=y2, in0=y_ps, scalar1=mv[:, 0:1],
                                    scalar2=rstd, op0=ALU.subtract,
                                    op1=ALU.mult)
            if apply_gb:
                nc.vector.tensor_mul(out=y2, in0=y2, in1=gb[0])
                nc.vector.tensor_add(out=y2, in0=y2, in1=gb[1])
            nc.sync.dma_start(out=y[qsl, :], in_=y2)


def build(apply_gb=True):
    nc = bacc.Bacc("TRN2", target_bir_lowering=False, debug=False,
                   enable_asserts=False, num_devices=N_CORES)
    ap = {}
    ap['xt8'] = nc.dram_tensor("xt8", [128, DC, T], FP8,
                               kind="ExternalInput").ap()
    ap['xq8'] = nc.dram_tensor("xq8", [QC, 128, D], BF16,
                               kind="ExternalInput").ap()
    ap['Wq'] = nc.dram_tensor("Wq", [128, DC, D], FP8,
                              kind="ExternalInput").ap()
    ap['bq'] = nc.dram_tensor("bq", [D], F32, kind="ExternalInput").ap()
    ap['Wk'] = nc.dram_tensor("Wk", [128, DC, 512], FP8,
                              kind="ExternalInput").ap()
    ap['bk'] = nc.dram_tensor("bk", [512], F32, kind="ExternalInput").ap()
    ap['Wv'] = nc.dram_tensor("Wv", [128, DC, 512], FP8,
                              kind="ExternalInput").ap()
    ap['bv'] = nc.dram_tensor("bv", [512], F32, kind="ExternalInput").ap()
    ap['Wo'] = nc.dram_tensor("Wo", [128, DC, D], FP8,
                              kind="ExternalInput").ap()
    ap['gamma'] = nc.dram_tensor("gamma", [D], F32, kind="ExternalInput").ap()
    ap['beta'] = nc.dram_tensor("beta", [D], F32, kind="ExternalInput").ap()
    ap['y'] = nc.dram_tensor("y", [TQ, D], F32, kind="ExternalOutput").ap()

    with tile.TileContext(nc) as tc, contextlib.ExitStack() as es:
        _body(nc, tc, ap, es, apply_gb)
    nc.compile()
    return nc


def _pack_rows(w):
    """[D, N] -> [128, DC, N] with rows (c*128+p) -> [p, c]."""
    n = w.shape[1]
    return np.ascontiguousarray(
        w.reshape(DC, 128, n).transpose(1, 0, 2))


def make_in_maps(inputs):
    """Per-core input maps; x token-rotated so q tokens come first."""
    f32 = {k: np.ascontiguousarray(np.asarray(v, dtype=np.float32))
           for k, v in inputs.items()}
    # slot order per core parity g: local heads (4g..4g+3) first, so slot s
    # holds canonical head (4g+s) mod 8 -> roll Wq/bq cols & Wo rows by -4g
    gshared = []
    for gg in range(2):
        r = -4 * gg * 128
        gshared.append({
            'Wq': _pack_rows(np.roll(WS * f32['Wq'], r, axis=1)).astype(E4),
            'Wo': _pack_rows(np.roll(WS * f32['Wo'], r, axis=0)).astype(E4),
            'bq': np.ascontiguousarray(np.roll(WS * f32['bq'], r)),
            'gamma': f32['gamma'], 'beta': f32['beta'],
        })
    wk8 = WS * f32['Wk']
    wv8 = WS * f32['Wv']
    x = f32['x']
    in_maps = []
    for core in range(N_CORES):
        b, gg = divmod(core, 2)
        own = slice(512 * gg, 512 * (gg + 1))
        xr = np.roll(x[b], -TQ * gg, axis=0)
        xq8 = (WS * (xr[:TQ] + f32['bo'])).astype(BF)
        in_maps.append({
            'xt8': _pack_rows(xr.T).astype(E4),
            'xq8': np.ascontiguousarray(xq8.reshape(QC, 128, D)),
            'Wk': _pack_rows(wk8[:, own]).astype(E4),
            'bk': WS * f32['bk'][own],
            'Wv': _pack_rows(wv8[:, own]).astype(E4),
            'bv': WS * f32['bv'][own],
            **gshared[gg]})
    return in_maps


_NC = {}


def kernel(**inputs):
    apply_gb = not (np.all(np.asarray(inputs['gamma']) == 1.0)
                    and np.all(np.asarray(inputs['beta']) == 0.0))
    in_maps = make_in_maps(inputs)
    if apply_gb not in _NC:
        _NC[apply_gb] = build(apply_gb)
    res = bass_utils.run_bass_kernel_spmd(_NC[apply_gb], in_maps,
                                          core_ids=list(range(N_CORES)))
    out = np.empty((B, T, D), dtype=np.float32)
    for core in range(N_CORES):
        b, gg = divmod(core, 2)
        out[b, TQ * gg:TQ * (gg + 1)] = res.results[core]['y']
    return out


# revision 24
# speedup vs baseline: 1.6948x; 1.0674x over previous
"""Multi-head attention block (QKV proj + softmax attention + out-proj +
residual + LayerNorm) on 8 TRN2 NeuronCores.

Sharding: core = (batch b, token-half g). Each core computes attention for
its 1024 query tokens over all 8 heads. K/V for the core's 4 local heads
are computed over the full 2048 tokens and exchanged with the pair partner
via AllGather; the gather latency hides under the V/Q projections.

Precision: weights are host-scaled by 8 and cast to fp8e4 (dodges fp8
subnormals; compensated exactly: exp scale /64 for Q*K, ones=8 for the
softmax denominator, LayerNorm scale-invariance with eps*64 for the
residual path). Matmuls with contraction >=256 run fp8 DoubleRow (2
contraction rows per pass); scores run bf16 (contraction = head dim 128).
The V projection runs with x as the stationary operand, producing
v[token, dh] directly (no PE transposes). The residual is injected into
the out-projection PSUM via an identity matmul. Accumulation is f32 in
PSUM; softmax statistics and LayerNorm are f32.
"""

import contextlib
import sys

if '/opt/trn_rl_repo' not in sys.path:
    sys.path.insert(0, '/opt/trn_rl_repo')

import ml_dtypes
import numpy as np

import concourse.bacc as bacc
import concourse.bass as bass
import concourse.bass_utils as bass_utils
import concourse.tile as tile
from concourse import mybir
from concourse.masks import make_identity

B, T, D, H = 4, 2048, 1024, 8
DH = 128            # head dim
TQ = T // 2         # query tokens per core
N_CORES = 8
DC = D // 128       # d-chunks of 128
KC = T // 128       # k-token chunks of 128
QC = TQ // 128      # q-token chunks of 128
EPS = 1e-5
WS = 8.0            # host-side weight scale (keeps fp8 weights normal)
SC_EXP = 1.0 / (float(np.sqrt(DH)) * WS * WS)
F32 = mybir.dt.float32
BF16 = mybir.dt.bfloat16
FP8 = mybir.dt.float8e4
AF = mybir.ActivationFunctionType
ALU = mybir.AluOpType
DR = mybir.MatmulPerfMode.DoubleRow
BF = ml_dtypes.bfloat16
E4 = ml_dtypes.float8_e4m3


def _body(nc, tc, ap, es, apply_gb):
    xt8, xq8, Wq, bq, Wk, bk, Wv, bv, Wo, gamma, beta, y = (
        ap['xt8'], ap['xq8'], ap['Wq'], ap['bq'], ap['Wk'], ap['bk'],
        ap['Wv'], ap['bv'], ap['Wo'], ap['gamma'], ap['beta'], ap['y'])

    consts = es.enter_context(tc.tile_pool(name="consts", bufs=1))
    w_pool = es.enter_context(tc.tile_pool(name="w", bufs=1))
    kt_pool = es.enter_context(tc.tile_pool(name="ktl", bufs=1))
    v_pool = es.enter_context(tc.tile_pool(name="vl", bufs=1))
    rem_pool = es.enter_context(tc.tile_pool(name="rem", bufs=1))
    qt_pool = es.enter_context(tc.tile_pool(name="qt", bufs=1))
    pt_pool = es.enter_context(tc.tile_pool(name="pt", bufs=2))
    cb_pool = es.enter_context(tc.tile_pool(name="cb", bufs=2))
    sums_pool = es.enter_context(tc.tile_pool(name="sums", bufs=2))
    y2_pool = es.enter_context(tc.tile_pool(name="y2", bufs=2))
    xq_pool = es.enter_context(tc.tile_pool(name="xq", bufs=1))
    dram = es.enter_context(tc.tile_pool(name="dram", bufs=1, space="DRAM"))

    # ---- weight / x loads (issue order = DMA priority) -------------------
    wk_t = w_pool.tile([128, DC, 512], FP8, tag="wk")
    nc.sync.dma_start(out=wk_t, in_=Wk)
    xt = w_pool.tile([128, DC, T], FP8, tag="xt")
    for tb in range(8):
        tsl = slice(tb * 256, (tb + 1) * 256)
        nc.sync.dma_start(out=xt[:, :, tsl], in_=xt8[:, :, tsl])
    wv_t = w_pool.tile([128, DC, 512], FP8, tag="wv")
    nc.sync.dma_start(out=wv_t, in_=Wv)
    wq_t = w_pool.tile([128, DC, D], FP8, tag="wq")
    nc.sync.dma_start(out=wq_t, in_=Wq)

    ident = consts.tile([128, 128], BF16, tag="ident")
    make_identity(nc, ident)
    ones = consts.tile([128, 2, 16], FP8, tag="ones")
    nc.vector.memset(ones, WS)
    eps_t = consts.tile([128, 1], F32, tag="eps")
    nc.vector.memset(eps_t, EPS * WS * WS)

    bq_t = consts.tile([128, H], F32, tag="bq")
    bk_t = consts.tile([128, 4], F32, tag="bk")
    nc.sync.dma_start(out=bq_t, in_=bq.rearrange("(h p) -> p h", p=128))
    nc.sync.dma_start(out=bk_t, in_=bk.rearrange("(h p) -> p h", p=128))

    def bcast128(name, src, n):
        t = consts.tile([128, n], F32, tag=name, name=name)
        src_b = bass.AP(tensor=src.tensor, offset=src.offset,
                        ap=[[0, 128]] + src.ap)
        nc.sync.dma_start(out=t, in_=src_b)
        return t

    bv_bc = bcast128("bv_bc", bv, 512)

    # late-phase tensors (prefetched mid-attention)
    wo_t = w_pool.tile([128, DC, D], FP8, tag="wo")
    xq_t = xq_pool.tile([128, QC, D], BF16, tag="xqs")
    ctx_all = w_pool.tile([128, QC, H, 128], FP8, tag="ctx_all")

    kt_loc = [kt_pool.tile([128, T], FP8, tag=f"ktl{j}", name=f"ktl{j}")
              for j in range(4)]
    v_loc4 = v_pool.tile([128, KC, 512], FP8, tag="v4")
    kt_rem = [rem_pool.tile([128, T], FP8, tag=f"ktr{j}", name=f"ktr{j}")
              for j in range(4)]
    v_rem = [rem_pool.tile([128, KC, 128], FP8, tag=f"vr{j}", name=f"vr{j}")
             for j in range(4)]
    k_send = dram.tile([4, 128, T], FP8, tag="k_send")
    k_sum = dram.tile([4, 128, T], FP8, tag="k_sum")
    v_send = dram.tile([4, 128, KC, 128], FP8, tag="v_send")
    v_sum = dram.tile([4, 128, KC, 128], FP8, tag="v_sum")

    def fetch_remote(j):
        # remote K/V = pair-sum minus local, recovered on the vector engine
        nc.sync.dma_start(out=kt_rem[j], in_=k_sum[j])
        nc.sync.dma_start(out=v_rem[j], in_=v_sum[j])
        nc.vector.tensor_tensor(out=kt_rem[j], in0=kt_rem[j],
                                in1=kt_loc[j], op=ALU.subtract)
        nc.vector.tensor_tensor(
            out=v_rem[j], in0=v_rem[j],
            in1=v_loc4[:, :, j * 128:(j + 1) * 128], op=ALU.subtract)

    with contextlib.ExitStack() as es2:
        proj_ps = es2.enter_context(tc.tile_pool(name="proj_ps", bufs=4,
                                                 space="PSUM"))

        # ---- K projection: 4 local heads over all T tokens --------------
        for j in range(4):
            jsl = slice(j * 128, (j + 1) * 128)
            for nt in range(T // 512):
                nsl = slice(nt * 512, (nt + 1) * 512)
                pp = proj_ps.tile([128, 512], F32, tag="ps", name="pp")
                for i in range(DC // 2):
                    nc.tensor.matmul(pp, wk_t[:, 2 * i:2 * i + 2, jsl],
                                     xt[:, 2 * i:2 * i + 2, nsl],
                                     start=(i == 0), stop=(i == DC // 2 - 1),
                                     perf_mode=DR)
                nc.vector.tensor_scalar(out=kt_loc[j][:, nsl], in0=pp,
                                        scalar1=bk_t[:, j:j + 1],
                                        scalar2=None, op0=ALU.add)
            nc.sync.dma_start(out=k_send[j], in_=kt_loc[j])
        nc.gpsimd.collective_compute(
            "AllReduce", ALU.add,
            ins=[k_send.opt()], outs=[k_sum.opt()],
            replica_groups=[[0, 1], [2, 3], [4, 5], [6, 7]])

        # ---- V projection (x stationary): per-chunk, all 4 local heads --
        for kc in range(KC):
            ksl = slice(kc * 128, (kc + 1) * 128)
            vp = proj_ps.tile([128, 512], F32, tag="ps", name="vp")
            for i in range(DC // 2):
                nc.tensor.matmul(vp, xt[:, 2 * i:2 * i + 2, ksl],
                                 wv_t[:, 2 * i:2 * i + 2, :],
                                 start=(i == 0), stop=(i == DC // 2 - 1),
                                 perf_mode=DR)
            nc.vector.tensor_tensor(out=v_loc4[:, kc, :], in0=vp,
                                    in1=bv_bc, op=ALU.add)
        for j in range(4):
            jsl = slice(j * 128, (j + 1) * 128)
            nc.sync.dma_start(out=v_send[j], in_=v_loc4[:, :, jsl])
        nc.gpsimd.collective_compute(
            "AllReduce", ALU.add,
            ins=[v_send.opt()], outs=[v_sum.opt()],
            replica_groups=[[0, 1], [2, 3], [4, 5], [6, 7]])

        # ---- Q projection: all 8 slots ----------------------------------
        qt = {}
        for h in range(H):
            hsl = slice(h * 128, (h + 1) * 128)
            qh = qt_pool.tile([128, TQ], BF16, tag=f"qt{h}", name=f"qt{h}")
            for nt in range(TQ // 512):
                nsl = slice(nt * 512, (nt + 1) * 512)
                qp = proj_ps.tile([128, 512], F32, tag="ps", name="qp")
                for i in range(DC // 2):
                    nc.tensor.matmul(qp, wq_t[:, 2 * i:2 * i + 2, hsl],
                                     xt[:, 2 * i:2 * i + 2, nsl],
                                     start=(i == 0), stop=(i == DC // 2 - 1),
                                     perf_mode=DR)
                nc.vector.tensor_scalar(out=qh[:, nsl], in0=qp,
                                        scalar1=bq_t[:, h:h + 1],
                                        scalar2=None, op0=ALU.add)
            qt[h] = qh

    # ---- attention ------------------------------------------------------
    with contextlib.ExitStack() as es3:
        s_psum = es3.enter_context(tc.tile_pool(name="s_ps", bufs=2,
                                                space="PSUM"))
        ctx_psum = es3.enter_context(tc.tile_pool(name="ctx_ps", bufs=1,
                                                  space="PSUM"))
        sum_psum = es3.enter_context(tc.tile_pool(name="sum_ps", bufs=1,
                                                  space="PSUM"))

        def scores_exp(h, pair):
            kt_h = kt_loc[h] if h < 4 else kt_rem[h - 4]
            qt_h = qt[h]
            pt = pt_pool.tile([128, 2, TQ], FP8, tag="pt", name="pt")
            for u in range(2):
                kc = 2 * pair + u
                ksl = slice(kc * 128, (kc + 1) * 128)
                s_ps = s_psum.tile([128, TQ], F32, tag="s", name="s_ps")
                for nq in range(TQ // 512):
                    nsl = slice(nq * 512, (nq + 1) * 512)
                    nc.tensor.matmul(s_ps[:, nsl], kt_h[:, ksl],
                                     qt_h[:, nsl], start=True, stop=True)
                nc.scalar.activation(out=pt[:, u, :], in_=s_ps,
                                     func=AF.Exp, scale=SC_EXP)
            return pt

        pt_cur = scores_exp(0, 0)
        for h in range(H):
            if h < 4:
                v_h = v_loc4[:, :, h * 128:(h + 1) * 128]
            else:
                v_h = v_rem[h - 4]
            if h == 1:
                nc.sync.dma_start(out=wo_t, in_=Wo)
            if h == 2:
                for qc2 in range(QC):
                    nc.sync.dma_start(out=xq_t[:, qc2, :], in_=xq8[qc2])
                gb = [bcast128("gamma_b", gamma, D),
                      bcast128("beta_b", beta, D)] if apply_gb else None

            ctx_ps = ctx_psum.tile([128, TQ], F32, tag="ctx_ps")
            sum_ps = sum_psum.tile([1, TQ], F32, tag="sum_ps")

            for pair in range(KC // 2):
                if pair + 1 < KC // 2:
                    pt_next = scores_exp(h, pair + 1)
                elif h + 1 < H:
                    pt_next = scores_exp(h + 1, 0)
                else:
                    pt_next = None
                first, last = (pair == 0), (pair == KC // 2 - 1)
                for nq in range(TQ // 512):
                    nsl = slice(nq * 512, (nq + 1) * 512)
                    nc.tensor.matmul(ctx_ps[:, nsl],
                                     v_h[:, 2 * pair:2 * pair + 2, :],
                                     pt_cur[:, :, nsl],
                                     start=first, stop=last, perf_mode=DR)
                for nq in range(TQ // 512):
                    nsl = slice(nq * 512, (nq + 1) * 512)
                    nc.tensor.matmul(sum_ps[:, nsl], ones[:, :, 0:1],
                                     pt_cur[:, :, nsl],
                                     start=first, stop=last, perf_mode=DR)
                pt_cur = pt_next

            # drain PSUM fast, normalize off the critical path
            ctx_bf = cb_pool.tile([128, TQ], BF16, tag="cbf")
            nc.vector.tensor_copy(out=ctx_bf, in_=ctx_ps)
            rsum = sums_pool.tile([1, TQ], F32, tag="rsum")
            nc.vector.reciprocal_approx_fast(out=rsum, in_=sum_ps)
            rsum_b = sums_pool.tile([128, TQ], F32, tag="rsum_b")
            nc.gpsimd.partition_broadcast(rsum_b, rsum, channels=128)
            nc.vector.tensor_tensor(
                out=ctx_all[:, :, h, :],
                in0=ctx_bf.rearrange("p (a c) -> p a c", c=128),
                in1=rsum_b.rearrange("p (a c) -> p a c", c=128),
                op=ALU.mult)
            if h < 4:
                fetch_remote(h)

    # ---- out-projection + residual + LayerNorm --------------------------
    with tc.tile_pool(name="y_ps", bufs=3, space="PSUM") as y_psum, \
            tc.tile_pool(name="ln", bufs=4) as ln_pool:
        for qc in range(QC):
            qsl = slice(qc * 128, (qc + 1) * 128)
            y_ps = y_psum.tile([128, D], F32, tag="y_ps")
            for no in range(D // 512):
                nsl = slice(no * 512, (no + 1) * 512)
                nc.tensor.matmul(y_ps[:, nsl], ident, xq_t[:, qc, nsl],
                                 start=True, stop=False)
                for i in range(H // 2):
                    nc.tensor.matmul(y_ps[:, nsl],
                                     ctx_all[:, qc, 2 * i:2 * i + 2, :],
                                     wo_t[:, 2 * i:2 * i + 2, nsl],
                                     start=False, stop=(i == H // 2 - 1),
                                     perf_mode=DR)

            stats = ln_pool.tile([128, 2, 6], F32, tag="stats")
            nc.vector.bn_stats(out=stats[:, 0, :], in_=y_ps[:, 0:512])
            nc.vector.bn_stats(out=stats[:, 1, :], in_=y_ps[:, 512:1024])
            mv = ln_pool.tile([128, 2], F32, tag="mv")
            nc.vector.bn_aggr(out=mv, in_=stats)
            std = ln_pool.tile([128, 1], F32, tag="std")
            nc.scalar.activation(out=std, in_=mv[:, 1:2], func=AF.Sqrt,
                                 bias=eps_t)
            rstd = ln_pool.tile([128, 1], F32, tag="rstd")
            nc.vector.reciprocal(out=rstd, in_=std)
            y2 = y2_pool.tile([128, D], F32, tag="y2")
            nc.vector.tensor_scalar(out=y2, in0=y_ps, scalar1=mv[:, 0:1],
                                    scalar2=rstd, op0=ALU.subtract,
                                    op1=ALU.mult)
            if apply_gb:
                nc.vector.tensor_mul(out=y2, in0=y2, in1=gb[0])
                nc.vector.tensor_add(out=y2, in0=y2, in1=gb[1])
            nc.sync.dma_start(out=y[qsl, :], in_=y2)


def build(apply_gb=True):
    nc = bacc.Bacc("TRN2", target_bir_lowering=False, debug=False,
                   enable_asserts=False, num_devices=N_CORES)
    ap = {}
    ap['xt8'] = nc.dram_tensor("xt8", [128, DC, T], FP8,
                               kind="ExternalInput").ap()
    ap['xq8'] = nc.dram_tensor("xq8", [QC, 128, D], BF16,
                               kind="ExternalInput").ap()
    ap['Wq'] = nc.dram_tensor("Wq", [128, DC, D], FP8,
                              kind="ExternalInput").ap()
    ap['bq'] = nc.dram_tensor("bq", [D], F32, kind="ExternalInput").ap()
    ap['Wk'] = nc.dram_tensor("Wk", [128, DC, 512], FP8,
                              kind="ExternalInput").ap()
    ap['bk'] = nc.dram_tensor("bk", [512], F32, kind="ExternalInput").ap()
    ap['Wv'] = nc.dram_tensor("Wv", [128, DC, 512], FP8,
                              kind="ExternalInput").ap()
    ap['bv'] = nc.dram_tensor("bv", [512], F32, kind="ExternalInput").ap()
    ap['Wo'] = nc.dram_tensor("Wo", [128, DC, D], FP8,
                              kind="ExternalInput").ap()
    ap['gamma'] = nc.dram_tensor("gamma", [D], F32, kind="ExternalInput").ap()
    ap['beta'] = nc.dram_tensor("beta", [D], F32, kind="ExternalInput").ap()
    ap['y'] = nc.dram_tensor("y", [TQ, D], F32, kind="ExternalOutput").ap()

    with tile.TileContext(nc) as tc, contextlib.ExitStack() as es:
        _body(nc, tc, ap, es, apply_gb)
    nc.compile()
    return nc


def _pack_rows(w):
    """[D, N] -> [128, DC, N] with rows (c*128+p) -> [p, c]."""
    n = w.shape[1]
    return np.ascontiguousarray(
        w.reshape(DC, 128, n).transpose(1, 0, 2))


def make_in_maps(inputs):
    """Per-core input maps; x token-rotated so q tokens come first."""
    f32 = {k: np.ascontiguousarray(np.asarray(v, dtype=np.float32))
           for k, v in inputs.items()}
    # slot order per core parity g: local heads (4g..4g+3) first, so slot s
    # holds canonical head (4g+s) mod 8 -> roll Wq/bq cols & Wo rows by -4g
    gshared = []
    for gg in range(2):
        r = -4 * gg * 128
        gshared.append({
            'Wq': _pack_rows(np.roll(WS * f32['Wq'], r, axis=1)).astype(E4),
            'Wo': _pack_rows(np.roll(WS * f32['Wo'], r, axis=0)).astype(E4),
            'bq': np.ascontiguousarray(np.roll(WS * f32['bq'], r)),
            'gamma': f32['gamma'], 'beta': f32['beta'],
        })
    wk8 = WS * f32['Wk']
    wv8 = WS * f32['Wv']
    x = f32['x']
    in_maps = []
    for core in range(N_CORES):
        b, gg = divmod(core, 2)
        own = slice(512 * gg, 512 * (gg + 1))
        xr = np.roll(x[b], -TQ * gg, axis=0)
        xq8 = (WS * (xr[:TQ] + f32['bo'])).astype(BF)
        in_maps.append({
            'xt8': _pack_rows(xr.T).astype(E4),
            'xq8': np.ascontiguousarray(xq8.reshape(QC, 128, D)),
            'Wk': _pack_rows(wk8[:, own]).astype(E4),
            'bk': WS * f32['bk'][own],
            'Wv': _pack_rows(wv8[:, own]).astype(E4),
            'bv': WS * f32['bv'][own],
            **gshared[gg]})
    return in_maps


_NC = {}


def kernel(**inputs):
    apply_gb = not (np.all(np.asarray(inputs['gamma']) == 1.0)
                    and np.all(np.asarray(inputs['beta']) == 0.0))
    in_maps = make_in_maps(inputs)
    if apply_gb not in _NC:
        _NC[apply_gb] = build(apply_gb)
    res = bass_utils.run_bass_kernel_spmd(_NC[apply_gb], in_maps,
                                          core_ids=list(range(N_CORES)))
    out = np.empty((B, T, D), dtype=np.float32)
    for core in range(N_CORES):
        b, gg = divmod(core, 2)
        out[b, TQ * gg:TQ * (gg + 1)] = res.results[core]['y']
    return out


# revision 29
# speedup vs baseline: 1.7158x; 1.0124x over previous
"""Multi-head attention block (QKV proj + softmax attention + out-proj +
residual + LayerNorm) on 8 TRN2 NeuronCores.

Sharding: core = (batch b, token-half g). Each core computes attention for
its 1024 query tokens over all 8 heads. K/V for the core's 4 local heads
are computed over the full 2048 tokens and exchanged with the pair partner
via AllGather; the gather latency hides under the V/Q projections.

Precision: weights are host-scaled by 8 and cast to fp8e4 (dodges fp8
subnormals; compensated exactly: exp scale /64 for Q*K, ones=8 for the
softmax denominator, LayerNorm scale-invariance with eps*64 for the
residual path). Matmuls with contraction >=256 run fp8 DoubleRow (2
contraction rows per pass); scores run bf16 (contraction = head dim 128).
The V projection runs with x as the stationary operand, producing
v[token, dh] directly (no PE transposes). The residual is injected into
the out-projection PSUM via an identity matmul. Accumulation is f32 in
PSUM; softmax statistics and LayerNorm are f32.
"""

import contextlib
import sys

if '/opt/trn_rl_repo' not in sys.path:
    sys.path.insert(0, '/opt/trn_rl_repo')

import ml_dtypes
import numpy as np

import concourse.bacc as bacc
import concourse.bass as bass
import concourse.bass_utils as bass_utils
import concourse.tile as tile
from concourse import mybir
from concourse.masks import make_identity

B, T, D, H = 4, 2048, 1024, 8
DH = 128            # head dim
TQ = T // 2         # query tokens per core
N_CORES = 8
DC = D // 128       # d-chunks of 128
KC = T // 128       # k-token chunks of 128
QC = TQ // 128      # q-token chunks of 128
EPS = 1e-5
WS = 8.0            # host-side weight scale (keeps fp8 weights normal)
SC_EXP = 1.0 / (float(np.sqrt(DH)) * WS * WS)
F32 = mybir.dt.float32
BF16 = mybir.dt.bfloat16
FP8 = mybir.dt.float8e4
AF = mybir.ActivationFunctionType
ALU = mybir.AluOpType
DR = mybir.MatmulPerfMode.DoubleRow
BF = ml_dtypes.bfloat16
E4 = ml_dtypes.float8_e4m3


def _body(nc, tc, ap, es, apply_gb):
    xt8, xq8, Wq, bq, Wk, bk, Wv, bv, Wo, gamma, beta, y = (
        ap['xt8'], ap['xq8'], ap['Wq'], ap['bq'], ap['Wk'], ap['bk'],
        ap['Wv'], ap['bv'], ap['Wo'], ap['gamma'], ap['beta'], ap['y'])

    consts = es.enter_context(tc.tile_pool(name="consts", bufs=1))
    w_pool = es.enter_context(tc.tile_pool(name="w", bufs=1))
    kt_pool = es.enter_context(tc.tile_pool(name="ktl", bufs=1))
    v_pool = es.enter_context(tc.tile_pool(name="vl", bufs=1))
    rem_pool = es.enter_context(tc.tile_pool(name="rem", bufs=1))
    qt_pool = es.enter_context(tc.tile_pool(name="qt", bufs=1))
    pt_pool = es.enter_context(tc.tile_pool(name="pt", bufs=2))
    cb_pool = es.enter_context(tc.tile_pool(name="cb", bufs=2))
    sums_pool = es.enter_context(tc.tile_pool(name="sums", bufs=2))
    y2_pool = es.enter_context(tc.tile_pool(name="y2", bufs=2))
    xq_pool = es.enter_context(tc.tile_pool(name="xq", bufs=1))
    dram = es.enter_context(tc.tile_pool(name="dram", bufs=1, space="DRAM"))

    # ---- weight / x loads (issue order = DMA priority) -------------------
    wv_t = w_pool.tile([128, DC, 512], FP8, tag="wv")
    nc.sync.dma_start(out=wv_t, in_=Wv)
    xt = w_pool.tile([128, DC, T], FP8, tag="xt")
    for tb in range(8):
        tsl = slice(tb * 256, (tb + 1) * 256)
        nc.sync.dma_start(out=xt[:, :, tsl], in_=xt8[:, :, tsl])
    wk_t = w_pool.tile([128, DC, 512], FP8, tag="wk")
    nc.sync.dma_start(out=wk_t, in_=Wk)
    wq_t = w_pool.tile([128, DC, D], FP8, tag="wq")
    nc.sync.dma_start(out=wq_t, in_=Wq)

    ident = consts.tile([128, 128], BF16, tag="ident")
    make_identity(nc, ident)
    ones = consts.tile([128, 2, 16], FP8, tag="ones")
    nc.vector.memset(ones, WS)
    eps_t = consts.tile([128, 1], F32, tag="eps")
    nc.vector.memset(eps_t, EPS * WS * WS)

    bq_t = consts.tile([128, H], F32, tag="bq")
    bk_t = consts.tile([128, 4], F32, tag="bk")
    nc.sync.dma_start(out=bq_t, in_=bq.rearrange("(h p) -> p h", p=128))
    nc.sync.dma_start(out=bk_t, in_=bk.rearrange("(h p) -> p h", p=128))

    def bcast128(name, src, n):
        t = consts.tile([128, n], F32, tag=name, name=name)
        src_b = bass.AP(tensor=src.tensor, offset=src.offset,
                        ap=[[0, 128]] + src.ap)
        nc.sync.dma_start(out=t, in_=src_b)
        return t

    bv_bc = bcast128("bv_bc", bv, 512)

    # late-phase tensors (prefetched mid-attention)
    wo_t = w_pool.tile([128, DC, D], FP8, tag="wo")
    xq_t = xq_pool.tile([128, QC, D], BF16, tag="xqs")
    ctx_all = w_pool.tile([128, QC, H, 128], FP8, tag="ctx_all")

    kt_loc = [kt_pool.tile([128, T], FP8, tag=f"ktl{j}", name=f"ktl{j}")
              for j in range(4)]
    v_loc4 = v_pool.tile([128, KC, 512], FP8, tag="v4")
    kt_rem = [rem_pool.tile([128, T], FP8, tag=f"ktr{j}", name=f"ktr{j}")
              for j in range(4)]
    v_rem = [rem_pool.tile([128, KC, 128], FP8, tag=f"vr{j}", name=f"vr{j}")
             for j in range(4)]
    tmp8_pool = es.enter_context(tc.tile_pool(name="tmp8", bufs=2))
    tmpb_pool = es.enter_context(tc.tile_pool(name="tmpb", bufs=2))
    k_send = dram.tile([4, 128, T], FP8, tag="k_send")
    k_all = dram.tile([2, 4, 128, T], FP8, tag="k_all")
    v_send = dram.tile([4, 128, KC, 128], FP8, tag="v_send")
    v_all = dram.tile([2, 4, 128, KC, 128], FP8, tag="v_all")

    def fetch_remote(j):
        # remote = (gathered blk0 + blk1) - local; rank-uniform, on vector
        b1 = tmp8_pool.tile([128, T], FP8, tag="b1", name="b1")
        nc.sync.dma_start(out=kt_rem[j], in_=k_all[0][j])
        nc.sync.dma_start(out=b1, in_=k_all[1][j])
        sbf = tmpb_pool.tile([128, T], BF16, tag="sbf", name="sbf")
        nc.vector.tensor_tensor(out=sbf, in0=kt_rem[j], in1=b1, op=ALU.add)
        nc.vector.tensor_tensor(out=kt_rem[j], in0=sbf, in1=kt_loc[j],
                                op=ALU.subtract)
        vb1 = tmp8_pool.tile([128, KC, 128], FP8, tag="vb1", name="vb1")
        nc.sync.dma_start(out=v_rem[j], in_=v_all[0][j])
        nc.sync.dma_start(out=vb1, in_=v_all[1][j])
        vbf = tmpb_pool.tile([128, KC, 128], BF16, tag="vbf", name="vbf")
        nc.vector.tensor_tensor(out=vbf, in0=v_rem[j], in1=vb1, op=ALU.add)
        nc.vector.tensor_tensor(
            out=v_rem[j], in0=vbf,
            in1=v_loc4[:, :, j * 128:(j + 1) * 128], op=ALU.subtract)

    with contextlib.ExitStack() as es2:
        proj_ps = es2.enter_context(tc.tile_pool(name="proj_ps", bufs=4,
                                                 space="PSUM"))

        # ---- V projection (x stationary): per-chunk, all 4 local heads --
        for kc in range(KC):
            ksl = slice(kc * 128, (kc + 1) * 128)
            vp = proj_ps.tile([128, 512], F32, tag="ps", name="vp")
            for i in range(DC // 2):
                nc.tensor.matmul(vp, xt[:, 2 * i:2 * i + 2, ksl],
                                 wv_t[:, 2 * i:2 * i + 2, :],
                                 start=(i == 0), stop=(i == DC // 2 - 1),
                                 perf_mode=DR)
            nc.vector.tensor_tensor(out=v_loc4[:, kc, :], in0=vp,
                                    in1=bv_bc, op=ALU.add)
        for j in range(4):
            jsl = slice(j * 128, (j + 1) * 128)
            nc.sync.dma_start(out=v_send[j], in_=v_loc4[:, :, jsl])
        nc.gpsimd.collective_compute(
            "AllGather", ALU.bypass,
            ins=[v_send.opt()], outs=[v_all.opt()],
            replica_groups=[[0, 1], [2, 3], [4, 5], [6, 7]])

        # ---- K projection: 4 local heads over all T tokens --------------
        for j in range(4):
            jsl = slice(j * 128, (j + 1) * 128)
            for nt in range(T // 512):
                nsl = slice(nt * 512, (nt + 1) * 512)
                pp = proj_ps.tile([128, 512], F32, tag="ps", name="pp")
                for i in range(DC // 2):
                    nc.tensor.matmul(pp, wk_t[:, 2 * i:2 * i + 2, jsl],
                                     xt[:, 2 * i:2 * i + 2, nsl],
                                     start=(i == 0), stop=(i == DC // 2 - 1),
                                     perf_mode=DR)
                nc.vector.tensor_scalar(out=kt_loc[j][:, nsl], in0=pp,
                                        scalar1=bk_t[:, j:j + 1],
                                        scalar2=None, op0=ALU.add)
            nc.sync.dma_start(out=k_send[j], in_=kt_loc[j])
        nc.gpsimd.collective_compute(
            "AllGather", ALU.bypass,
            ins=[k_send.opt()], outs=[k_all.opt()],
            replica_groups=[[0, 1], [2, 3], [4, 5], [6, 7]])

        # ---- Q projection: all 8 slots ----------------------------------
        qt = {}
        for h in range(H):
            hsl = slice(h * 128, (h + 1) * 128)
            qh = qt_pool.tile([128, TQ], BF16, tag=f"qt{h}", name=f"qt{h}")
            for nt in range(TQ // 512):
                nsl = slice(nt * 512, (nt + 1) * 512)
                qp = proj_ps.tile([128, 512], F32, tag="ps", name="qp")
                for i in range(DC // 2):
                    nc.tensor.matmul(qp, wq_t[:, 2 * i:2 * i + 2, hsl],
                                     xt[:, 2 * i:2 * i + 2, nsl],
                                     start=(i == 0), stop=(i == DC // 2 - 1),
                                     perf_mode=DR)
                nc.vector.tensor_scalar(out=qh[:, nsl], in0=qp,
                                        scalar1=bq_t[:, h:h + 1],
                                        scalar2=None, op0=ALU.add)
            qt[h] = qh

    # ---- attention ------------------------------------------------------
    with contextlib.ExitStack() as es3:
        s_psum = es3.enter_context(tc.tile_pool(name="s_ps", bufs=2,
                                                space="PSUM"))
        ctx_psum = es3.enter_context(tc.tile_pool(name="ctx_ps", bufs=1,
                                                  space="PSUM"))
        sum_psum = es3.enter_context(tc.tile_pool(name="sum_ps", bufs=1,
                                                  space="PSUM"))

        def scores_exp(h, pair):
            kt_h = kt_loc[h] if h < 4 else kt_rem[h - 4]
            qt_h = qt[h]
            pt = pt_pool.tile([128, 2, TQ], FP8, tag="pt", name="pt")
            for u in range(2):
                kc = 2 * pair + u
                ksl = slice(kc * 128, (kc + 1) * 128)
                s_ps = s_psum.tile([128, TQ], F32, tag="s", name="s_ps")
                for nq in range(TQ // 512):
                    nsl = slice(nq * 512, (nq + 1) * 512)
                    nc.tensor.matmul(s_ps[:, nsl], kt_h[:, ksl],
                                     qt_h[:, nsl], start=True, stop=True)
                nc.scalar.activation(out=pt[:, u, :], in_=s_ps,
                                     func=AF.Exp, scale=SC_EXP)
            return pt

        pt_cur = scores_exp(0, 0)
        for h in range(H):
            if h < 4:
                v_h = v_loc4[:, :, h * 128:(h + 1) * 128]
            else:
                v_h = v_rem[h - 4]
            if h == 1:
                nc.sync.dma_start(out=wo_t, in_=Wo)
            if h == 2:
                for qc2 in range(QC):
                    nc.sync.dma_start(out=xq_t[:, qc2, :], in_=xq8[qc2])
                gb = [bcast128("gamma_b", gamma, D),
                      bcast128("beta_b", beta, D)] if apply_gb else None
                fetch_remote(0)
                fetch_remote(1)
            if h == 3:
                fetch_remote(2)
                fetch_remote(3)

            ctx_ps = ctx_psum.tile([128, TQ], F32, tag="ctx_ps")
            sum_ps = sum_psum.tile([1, TQ], F32, tag="sum_ps")

            for pair in range(KC // 2):
                if pair + 1 < KC // 2:
                    pt_next = scores_exp(h, pair + 1)
                elif h + 1 < H:
                    pt_next = scores_exp(h + 1, 0)
                else:
                    pt_next = None
                first, last = (pair == 0), (pair == KC // 2 - 1)
                for nq in range(TQ // 512):
                    nsl = slice(nq * 512, (nq + 1) * 512)
                    nc.tensor.matmul(ctx_ps[:, nsl],
                                     v_h[:, 2 * pair:2 * pair + 2, :],
                                     pt_cur[:, :, nsl],
                                     start=first, stop=last, perf_mode=DR)
                for nq in range(TQ // 512):
                    nsl = slice(nq * 512, (nq + 1) * 512)
                    nc.tensor.matmul(sum_ps[:, nsl], ones[:, :, 0:1],
                                     pt_cur[:, :, nsl],
                                     start=first, stop=last, perf_mode=DR)
                pt_cur = pt_next

            # drain PSUM fast, normalize off the critical path
            ctx_bf = cb_pool.tile([128, TQ], BF16, tag="cbf")
            nc.vector.tensor_copy(out=ctx_bf, in_=ctx_ps)
            rsum = sums_pool.tile([1, TQ], F32, tag="rsum")
            nc.vector.reciprocal_approx_fast(out=rsum, in_=sum_ps)
            rsum_b = sums_pool.tile([128, TQ], F32, tag="rsum_b")
            nc.gpsimd.partition_broadcast(rsum_b, rsum, channels=128)
            nc.vector.tensor_tensor(
                out=ctx_all[:, :, h, :],
                in0=ctx_bf.rearrange("p (a c) -> p a c", c=128),
                in1=rsum_b.rearrange("p (a c) -> p a c", c=128),
                op=ALU.mult)

    # ---- out-projection + residual + LayerNorm --------------------------
    with tc.tile_pool(name="y_ps", bufs=3, space="PSUM") as y_psum, \
            tc.tile_pool(name="ln", bufs=4) as ln_pool:
        for qc in range(QC):
            qsl = slice(qc * 128, (qc + 1) * 128)
            y_ps = y_psum.tile([128, D], F32, tag="y_ps")
            for no in range(D // 512):
                nsl = slice(no * 512, (no + 1) * 512)
                nc.tensor.matmul(y_ps[:, nsl], ident, xq_t[:, qc, nsl],
                                 start=True, stop=False)
                for i in range(H // 2):
                    nc.tensor.matmul(y_ps[:, nsl],
                                     ctx_all[:, qc, 2 * i:2 * i + 2, :],
                                     wo_t[:, 2 * i:2 * i + 2, nsl],
                                     start=False, stop=(i == H // 2 - 1),
                                     perf_mode=DR)

            stats = ln_pool.tile([128, 2, 6], F32, tag="stats")
            nc.vector.bn_stats(out=stats[:, 0, :], in_=y_ps[:, 0:512])
            nc.vector.bn_stats(out=stats[:, 1, :], in_=y_ps[:, 512:1024])
            mv = ln_pool.tile([128, 2], F32, tag="mv")
            nc.vector.bn_aggr(out=mv, in_=stats)
            std = ln_pool.tile([128, 1], F32, tag="std")
            nc.scalar.activation(out=std, in_=mv[:, 1:2], func=AF.Sqrt,
                                 bias=eps_t)
            rstd = ln_pool.tile([128, 1], F32, tag="rstd")
            nc.vector.reciprocal(out=rstd, in_=std)
            y2 = y2_pool.tile([128, D], F32, tag="y2")
            nc.vector.tensor_scalar(out=y2, in0=y_ps, scalar1=mv[:, 0:1],
                                    scalar2=rstd, op0=ALU.subtract,
                                    op1=ALU.mult)
            if apply_gb:
                nc.vector.tensor_mul(out=y2, in0=y2, in1=gb[0])
                nc.vector.tensor_add(out=y2, in0=y2, in1=gb[1])
            nc.sync.dma_start(out=y[qsl, :], in_=y2)


def build(apply_gb=True):
    nc = bacc.Bacc("TRN2", target_bir_lowering=False, debug=False,
                   enable_asserts=False, num_devices=N_CORES)
    ap = {}
    ap['xt8'] = nc.dram_tensor("xt8", [128, DC, T], FP8,
                               kind="ExternalInput").ap()
    ap['xq8'] = nc.dram_tensor("xq8", [QC, 128, D], BF16,
                               kind="ExternalInput").ap()
    ap['Wq'] = nc.dram_tensor("Wq", [128, DC, D], FP8,
                              kind="ExternalInput").ap()
    ap['bq'] = nc.dram_tensor("bq", [D], F32, kind="ExternalInput").ap()
    ap['Wk'] = nc.dram_tensor("Wk", [128, DC, 512], FP8,
                              kind="ExternalInput").ap()
    ap['bk'] = nc.dram_tensor("bk", [512], F32, kind="ExternalInput").ap()
    ap['Wv'] = nc.dram_tensor("Wv", [128, DC, 512], FP8,
                              kind="ExternalInput").ap()
    ap['bv'] = nc.dram_tensor("bv", [512], F32, kind="ExternalInput").ap()
    ap['Wo'] = nc.dram_tensor("Wo", [128, DC, D], FP8,
                              kind="ExternalInput").ap()
    ap['gamma'] = nc.dram_tensor("gamma", [D], F32, kind="ExternalInput").ap()
    ap['beta'] = nc.dram_tensor("beta", [D], F32, kind="ExternalInput").ap()
    ap['y'] = nc.dram_tensor("y", [TQ, D], F32, kind="ExternalOutput").ap()

    with tile.TileContext(nc) as tc, contextlib.ExitStack() as es:
        _body(nc, tc, ap, es, apply_gb)
    nc.compile()
    return nc


def _pack_rows(w):
    """[D, N] -> [128, DC, N] with rows (c*128+p) -> [p, c]."""
    n = w.shape[1]
    return np.ascontiguousarray(
        w.reshape(DC, 128, n).transpose(1, 0, 2))


def make_in_maps(inputs):
    """Per-core input maps; x token-rotated so q tokens come first."""
    f32 = {k: np.ascontiguousarray(np.asarray(v, dtype=np.float32))
           for k, v in inputs.items()}
    # slot order per core parity g: local heads (4g..4g+3) first, so slot s
    # holds canonical head (4g+s) mod 8 -> roll Wq/bq cols & Wo rows by -4g
    gshared = []
    for gg in range(2):
        r = -4 * gg * 128
        gshared.append({
            'Wq': _pack_rows(np.roll(WS * f32['Wq'], r, axis=1)).astype(E4),
            'Wo': _pack_rows(np.roll(WS * f32['Wo'], r, axis=0)).astype(E4),
            'bq': np.ascontiguousarray(np.roll(WS * f32['bq'], r)),
            'gamma': f32['gamma'], 'beta': f32['beta'],
        })
    wk8 = WS * f32['Wk']
    wv8 = WS * f32['Wv']
    x = f32['x']
    in_maps = []
    for core in range(N_CORES):
        b, gg = divmod(core, 2)
        own = slice(512 * gg, 512 * (gg + 1))
        xr = np.roll(x[b], -TQ * gg, axis=0)
        xq8 = (WS * (xr[:TQ] + f32['bo'])).astype(BF)
        in_maps.append({
            'xt8': _pack_rows(xr.T).astype(E4),
            'xq8': np.ascontiguousarray(xq8.reshape(QC, 128, D)),
            'Wk': _pack_rows(wk8[:, own]).astype(E4),
            'bk': WS * f32['bk'][own],
            'Wv': _pack_rows(wv8[:, own]).astype(E4),
            'bv': WS * f32['bv'][own],
            **gshared[gg]})
    return in_maps


_NC = {}


def kernel(**inputs):
    apply_gb = not (np.all(np.asarray(inputs['gamma']) == 1.0)
                    and np.all(np.asarray(inputs['beta']) == 0.0))
    in_maps = make_in_maps(inputs)
    if apply_gb not in _NC:
        _NC[apply_gb] = build(apply_gb)
    res = bass_utils.run_bass_kernel_spmd(_NC[apply_gb], in_maps,
                                          core_ids=list(range(N_CORES)))
    out = np.empty((B, T, D), dtype=np.float32)
    for core in range(N_CORES):
        b, gg = divmod(core, 2)
        out[b, TQ * gg:TQ * (gg + 1)] = res.results[core]['y']
    return out
